# revision 1
# baseline (speedup 1.0000x reference)
"""GQA (32 q heads / 8 kv heads, head_dim 64, causal, QK-RMSNorm + RoPE) on 8 TRN2 cores.

Sharding: data-parallel over batch (2) x tensor-parallel over heads (4):
each core handles one batch element, 8 query heads, 2 kv heads, and produces
a partial output (its heads' slice of the Wo contraction); the host sums the
4 partials per batch element.

On-chip layout is "transposed": activations live as x^T / q^T / k^T with the
feature dim on partitions and tokens on the free dim, so Q/K/V/O projections
run with natural weight layouts and the softmax reduction (over keys) lands on
the PE via a ones-column appended to V (denominator accumulated in the same
matmul as the attention output).  All matmuls run in float32r.
"""

import numpy as np

import concourse.bass as bass
import concourse.mybir as mybir
import concourse.tile as tile
from concourse import bacc
from concourse.bass_utils import run_bass_kernel_spmd

# Problem config (hardcoded per contract)
B, T, D = 2, 2048, 2048
H, KV, HD = 32, 8, 64
GROUPS = H // KV
THETA = 10000.0
SCALE = 1.0 / np.sqrt(HD)
EPS = 1e-6

# Per-core sharding
HQL = H // 4          # 8 local q heads
KVL = KV // 4         # 2 local kv heads
FQ = HQL * HD         # 512
FKV = KVL * HD        # 128

# Tiling
P = 128
TB = 512              # token block
NTB = T // TB         # 4
NDC = D // P          # 16 contraction chunks
NKC = T // P          # 16 key chunks
NQC = FQ // P         # 4 q-proj chunks (2 heads each)

f32 = mybir.dt.float32
f32r = mybir.dt.float32r
AF = mybir.ActivationFunctionType
ALU = mybir.AluOpType


def _build_nc():
    nc = bacc.Bacc("TRN2", target_bir_lowering=False, debug=False, num_devices=8)

    eps_t = nc.alloc_sbuf_tensor("const-f32-eps", [128, 1], f32)
    nc.gpsimd.memset(eps_t.ap(), EPS)
    nc.const_aps.aps[(f32, EPS)] = eps_t.ap()
    nc.all_engine_barrier()

    xT_d = nc.dram_tensor("xT", [D, T], f32r, kind="ExternalInput")
    wq_d = nc.dram_tensor("wq", [D, FQ], f32r, kind="ExternalInput")
    wk_d = nc.dram_tensor("wk", [D, FKV], f32r, kind="ExternalInput")
    wv_d = nc.dram_tensor("wv", [D, FKV], f32r, kind="ExternalInput")
    wo_d = nc.dram_tensor("wo", [FQ, D], f32r, kind="ExternalInput")
    cosq_d = nc.dram_tensor("cosq", [P, T], f32, kind="ExternalInput")
    cosk_d = nc.dram_tensor("cosk", [P, T], f32, kind="ExternalInput")
    sin_d = nc.dram_tensor("sin", [P, T], f32, kind="ExternalInput")
    rqT_d = nc.dram_tensor("rqT", [P, P], f32r, kind="ExternalInput")
    rkT_d = nc.dram_tensor("rkT", [P, P], f32r, kind="ExternalInput")
    hsel_d = nc.dram_tensor("hsel", [P, 2], f32r, kind="ExternalInput")
    hexp_d = nc.dram_tensor("hexp", [2, P], f32r, kind="ExternalInput")
    e1_d = nc.dram_tensor("e1", [1, P], f32r, kind="ExternalInput")
    masks_d = nc.dram_tensor("masks", [P, 4, TB], f32r, kind="ExternalInput")
    ident_d = nc.dram_tensor("ident", [P, P], f32r, kind="ExternalInput")
    outT_d = nc.dram_tensor("outT", [D, T], f32, kind="ExternalOutput")

    with tile.TileContext(nc) as tc:
        with (
            tc.tile_pool(name="wpool", bufs=1) as wpool,
            tc.tile_pool(name="cpool", bufs=1) as cpool,
            tc.tile_pool(name="kvpool", bufs=1) as kvpool,
            tc.tile_pool(name="trig", bufs=1) as trig,
            tc.tile_pool(name="xpool", bufs=5) as xpool,
            tc.tile_pool(name="qpool", bufs=1) as qpool,
            tc.tile_pool(name="btmp", bufs=2) as btmp,
            tc.tile_pool(name="spool", bufs=2) as spool,
            tc.tile_pool(name="epool", bufs=6) as epool,
            tc.tile_pool(name="opool", bufs=1) as opool,
            tc.tile_pool(name="outp", bufs=2) as outp,
            tc.tile_pool(name="psum", bufs=6, space="PSUM") as psum,
            tc.tile_pool(name="psmall", bufs=2, space="PSUM") as psmall,
        ):
            # ---- persistent weights / constants ----
            wq_sb = wpool.tile([P, NDC, FQ], f32r)
            wk_sb = wpool.tile([P, NDC, FKV], f32r)
            wv_sb = wpool.tile([P, NDC, FKV], f32r)
            wo_sb = wpool.tile([P, NQC, D], f32r)
            nc.sync.dma_start(wq_sb[:], wq_d.rearrange("(ko p) f -> p ko f", p=P))
            nc.sync.dma_start(wk_sb[:], wk_d.rearrange("(ko p) f -> p ko f", p=P))
            nc.sync.dma_start(wv_sb[:], wv_d.rearrange("(ko p) f -> p ko f", p=P))
            nc.sync.dma_start(wo_sb[:], wo_d.rearrange("(ko p) f -> p ko f", p=P))

            rqT_sb = cpool.tile([P, P], f32r)
            rkT_sb = cpool.tile([P, P], f32r)
            hsel_sb = cpool.tile([P, 2], f32r)
            hexp_sb = cpool.tile([2, P], f32r)
            e1_sb = cpool.tile([1, P], f32r)
            masks_sb = cpool.tile([P, 4, TB], f32r)
            ident_sb = cpool.tile([P, P], f32r)
            nc.sync.dma_start(rqT_sb[:], rqT_d[:])
            nc.sync.dma_start(rkT_sb[:], rkT_d[:])
            nc.sync.dma_start(hsel_sb[:], hsel_d[:])
            nc.sync.dma_start(hexp_sb[:], hexp_d[:])
            nc.sync.dma_start(e1_sb[:], e1_d[:])
            nc.sync.dma_start(masks_sb[:], masks_d[:])
            nc.sync.dma_start(ident_sb[:], ident_d[:])

            # K^T (per-kv-head at both partition halves) and V (+ones col)
            ktf = kvpool.tile([P, T], f32r)          # rows 0:64 kv0, 64:128 kv1
            kts = kvpool.tile([P, T], f32r)          # swapped halves
            v_sb = kvpool.tile([P, NKC, KVL, 66], f32r)  # [tok, kc, g, hd+ones+pad]
            ones_bc = nc.const_aps.tensor(1.0, (P, NKC, KVL, 66), f32)
            nc.vector.tensor_copy(v_sb[:], ones_bc)

            for tb in range(NTB):
                tbs = slice(tb * TB, (tb + 1) * TB)

                cq_t = trig.tile([P, TB], f32, tag="cq")
                ck_t = trig.tile([P, TB], f32, tag="ck")
                sn_t = trig.tile([P, TB], f32, tag="sn")
                nc.sync.dma_start(cq_t[:], cosq_d[:, tbs])
                nc.sync.dma_start(ck_t[:], cosk_d[:, tbs])
                nc.sync.dma_start(sn_t[:], sin_d[:, tbs])

                # ---- A: projections ----
                qps = [psum.tile([P, TB], f32, tag="big", name=f"qps{_f}") for _f in range(NQC)]
                kps = psum.tile([P, TB], f32, tag="big")
                vps = psum.tile([P, TB], f32, tag="big")
                for dc in range(NDC):
                    xt = xpool.tile([P, TB], f32r)
                    nc.sync.dma_start(xt[:], xT_d[dc * P:(dc + 1) * P, tbs])
                    st = dc == 0
                    sp = dc == NDC - 1
                    for fc in range(NQC):
                        nc.tensor.matmul(qps[fc][:], wq_sb[:, dc, fc * P:(fc + 1) * P],
                                         xt[:], start=st, stop=sp)
                    nc.tensor.matmul(kps[:], wk_sb[:, dc, :], xt[:], start=st, stop=sp)
                    nc.tensor.matmul(vps[:], wv_sb[:, dc, :], xt[:], start=st, stop=sp)

                # ---- B: RMSNorm + RoPE on Q chunks and K ----
                qts = []
                for ci in range(NQC + 1):
                    is_k = ci == NQC
                    cps = kps if is_k else qps[ci]
                    rT = rkT_sb if is_k else rqT_sb
                    ct = ck_t if is_k else cq_t

                    qsb = btmp.tile([P, TB], f32r, tag="qsb")
                    nc.vector.tensor_copy(qsb[:], cps[:])
                    sq = btmp.tile([P, TB], f32r, tag="sq")
                    nc.scalar.square(sq[:], cps[:])
                    ss = psmall.tile([2, TB], f32, tag="sps", name="ss")
                    nc.tensor.matmul(ss[:], hsel_sb[:], sq[:], start=True, stop=True)
                    lnb = spool.tile([2, TB], f32, tag="lnb")
                    nc.scalar.activation(lnb[:], ss[:], AF.Ln, bias=EPS, scale=1.0 / HD)
                    rr = spool.tile([2, TB], f32r, tag="rr")
                    nc.scalar.activation(rr[:], lnb[:], AF.Exp, scale=-0.5)
                    bc = psum.tile([P, TB], f32, tag="big")
                    nc.tensor.matmul(bc[:], hexp_sb[:], rr[:], start=True, stop=True)
                    rot = psum.tile([P, TB], f32, tag="big")
                    nc.tensor.matmul(rot[:], rT[:], qsb[:], start=True, stop=True)
                    m1 = btmp.tile([P, TB], f32, tag="m1")
                    nc.vector.tensor_tensor(m1[:], qsb[:], ct[:], ALU.mult)
                    m2 = btmp.tile([P, TB], f32, tag="m2")
                    nc.vector.tensor_tensor(m2[:], rot[:], sn_t[:], ALU.mult)
                    s12 = btmp.tile([P, TB], f32, tag="m1", name="s12")
                    nc.vector.tensor_tensor(s12[:], m1[:], m2[:], ALU.add)
                    if not is_k:
                        qt = qpool.tile([P, TB], f32r, tag=f"qt{ci}")
                        nc.vector.tensor_tensor(qt[:], s12[:], bc[:], ALU.mult)
                        qts.append(qt)
                    else:
                        nc.vector.tensor_tensor(ktf[:, tbs], s12[:], bc[:], ALU.mult)
                        nc.vector.tensor_tensor(kts[0:64, tbs], s12[64:P], bc[64:P], ALU.mult)
                        nc.vector.tensor_tensor(kts[64:P, tbs], s12[0:64], bc[0:64], ALU.mult)

                # ---- C: V transpose into [tok, hd] with ones column ----
                vt_sb = btmp.tile([P, TB], f32r, tag="sq", name="vt_sb")
                nc.vector.tensor_copy(vt_sb[:], vps[:])
                for st4 in range(TB // P):
                    kc = tb * (TB // P) + st4
                    tp = psum.tile([P, P], f32r, tag="big")
                    nc.tensor.transpose(tp[:], vt_sb[:, st4 * P:(st4 + 1) * P], ident_sb[:])
                    nc.vector.tensor_copy(v_sb[:, kc, 0, 0:64], tp[:, 0:64])
                    nc.vector.tensor_copy(v_sb[:, kc, 1, 0:64], tp[:, 64:P])

                # ---- D: attention for query block tb ----
                nkc = (tb + 1) * (TB // P)
                for g in range(KVL):
                    o_ps = [psum.tile([65, TB], f32, tag="big", name=f"ops{_h}") for _h in range(GROUPS)]
                    for kc in range(nkc):
                        kslice = slice(kc * P, (kc + 1) * P)
                        for hj in range(GROUPS):
                            hl = GROUPS * g + hj
                            bq = 64 * (hl % 2)
                            cf = hl // 2
                            kt_tile = ktf if bq == 64 * g else kts
                            sps = psmall.tile([P, TB], f32, tag="sps", name="sps")
                            nc.tensor.matmul(sps[:], kt_tile[bq:bq + 64, kslice],
                                             qts[cf][bq:bq + 64, :], start=True, stop=True)
                            es = epool.tile([P, TB], f32r, tag="es")
                            nc.scalar.activation(es[:], sps[:], AF.Exp, scale=float(SCALE))
                            tdiag = kc - tb * (TB // P)
                            if tdiag >= 0:
                                nc.vector.tensor_tensor(es[:], es[:], masks_sb[:, tdiag, :], ALU.mult)
                            nc.tensor.matmul(o_ps[hj][:], v_sb[:, kc, g, 0:65], es[:],
                                             start=(kc == 0), stop=(kc == nkc - 1))
                    # normalize + pack head pairs for the O projection
                    for pj in range(2):
                        cf = 2 * g + pj
                        hA = 2 * pj
                        hB = hA + 1
                        ldA = spool.tile([1, TB], f32, tag="ld", name="ldA")
                        nc.scalar.activation(ldA[:], o_ps[hA][64:65, :], AF.Ln)
                        ldB = spool.tile([1, TB], f32, tag="ld", name="ldB")
                        nc.scalar.activation(ldB[:], o_ps[hB][64:65, :], AF.Ln)
                        rpA = spool.tile([1, TB], f32r, tag="rp", name="rpA")
                        nc.scalar.activation(rpA[:], ldA[:], AF.Exp, scale=-1.0)
                        rpB = spool.tile([1, TB], f32r, tag="rp", name="rpB")
                        nc.scalar.activation(rpB[:], ldB[:], AF.Exp, scale=-1.0)
                        bc2 = psum.tile([P, TB], f32, tag="big")
                        nc.tensor.matmul(bc2[:], hexp_sb[0:1, :], rpA[:], start=True, stop=False)
                        nc.tensor.matmul(bc2[:], e1_sb[:], rpB[:], start=False, stop=True)
                        osb = opool.tile([P, TB], f32, tag="osb")
                        nc.vector.tensor_copy(osb[0:64, :], o_ps[hA][0:64, :])
                        nc.vector.tensor_copy(osb[64:P, :], o_ps[hB][0:64, :])
                        orhs = opool.tile([P, TB], f32r, tag=f"orhs{cf}")
                        nc.vector.tensor_tensor(orhs[:], osb[:], bc2[:], ALU.mult)
                        if g == 0 and pj == 0:
                            orhs_list = [None] * NQC
                        orhs_list[cf] = orhs

                # ---- E: output projection for this token block ----
                for dc2 in range(NDC):
                    ops_ = psum.tile([P, TB], f32, tag="big")
                    for cf in range(NQC):
                        nc.tensor.matmul(ops_[:], wo_sb[:, cf, dc2 * P:(dc2 + 1) * P],
                                         orhs_list[cf][:], start=(cf == 0), stop=(cf == NQC - 1))
                    ob = outp.tile([P, TB], f32, tag="ob")
                    nc.vector.tensor_copy(ob[:], ops_[:])
                    nc.sync.dma_start(outT_d[dc2 * P:(dc2 + 1) * P, tbs], ob[:])

    nc.compile()
    return nc


_NC_CACHE = None


def _get_nc():
    global _NC_CACHE
    if _NC_CACHE is None:
        _NC_CACHE = _build_nc()
    return _NC_CACHE


def _host_constants(q_scale, k_scale):
    pos = np.arange(T, dtype=np.float64)
    invf = 1.0 / (THETA ** (np.arange(0, HD, 2, dtype=np.float64) / HD))  # (32,)
    ang = pos[:, None] * invf[None, :]                                    # (T, 32)
    c = np.cos(ang)
    s = np.sin(ang)
    pidx = np.arange(P) % 32
    hidx = np.arange(P) % HD
    cosq = (c[:, pidx].T * q_scale[hidx][:, None]).astype(np.float32)     # (128, T)
    cosk = (c[:, pidx].T * k_scale[hidx][:, None]).astype(np.float32)
    sin = s[:, pidx].T.astype(np.float32)

    def rmat(scale):
        R = np.zeros((HD, HD), dtype=np.float64)
        for i in range(32):
            R[i, i + 32] = -scale[i + 32]
            R[i + 32, i] = scale[i]
        M = np.kron(np.eye(2), R)
        return np.ascontiguousarray(M.T.astype(np.float32))

    hsel = np.zeros((P, 2), dtype=np.float32)
    hsel[0:64, 0] = 1.0
    hsel[64:P, 1] = 1.0
    hexp = np.ascontiguousarray(hsel.T)

    masks = np.zeros((P, 4, TB), dtype=np.float32)
    pp = np.arange(P)[:, None]
    ff = np.arange(TB)[None, :]
    for t in range(4):
        masks[:, t, :] = (ff >= pp + P * t).astype(np.float32)

    ident = np.eye(P, dtype=np.float32)
    return cosq, cosk, sin, rmat(q_scale), rmat(k_scale), hsel, hexp, masks, ident


def _run(inputs, trace=False):
    x = np.asarray(inputs["x"], dtype=np.float32)
    Wq = np.asarray(inputs["Wq"], dtype=np.float32)
    Wk = np.asarray(inputs["Wk"], dtype=np.float32)
    Wv = np.asarray(inputs["Wv"], dtype=np.float32)
    Wo = np.asarray(inputs["Wo"], dtype=np.float32)
    q_scale = np.asarray(inputs["q_scale"], dtype=np.float64)
    k_scale = np.asarray(inputs["k_scale"], dtype=np.float64)

    cosq, cosk, sin, rqT, rkT, hsel, hexp, masks, ident = _host_constants(q_scale, k_scale)

    in_maps = []
    for c in range(8):
        b = c // 4
        r = c % 4
        in_maps.append({
            "xT": np.ascontiguousarray(x[b].T),
            "wq": np.ascontiguousarray(Wq[:, r * FQ:(r + 1) * FQ]),
            "wk": np.ascontiguousarray(Wk[:, r * FKV:(r + 1) * FKV]),
            "wv": np.ascontiguousarray(Wv[:, r * FKV:(r + 1) * FKV]),
            "wo": np.ascontiguousarray(Wo[r * FQ:(r + 1) * FQ, :]),
            "cosq": cosq, "cosk": cosk, "sin": sin,
            "rqT": rqT, "rkT": rkT, "hsel": hsel, "hexp": hexp,
            "e1": np.ascontiguousarray(hexp[1:2, :]),
            "masks": masks, "ident": ident,
        })

    nc = _get_nc()
    res = run_bass_kernel_spmd(nc, in_maps, core_ids=list(range(8)), trace=trace)
    out = np.empty((B, T, D), dtype=np.float32)
    for b in range(B):
        acc = res.results[4 * b]["outT"].astype(np.float32).copy()
        for r in range(1, 4):
            acc += res.results[4 * b + r]["outT"]
        out[b] = acc.T
    return out, res


def kernel(**inputs):
    out, _ = _run(inputs, trace=False)
    return out



# revision 19
# speedup vs baseline: 1.4173x; 1.4173x over previous
"""GQA (32 q heads / 8 kv heads, head_dim 64, causal, QK-RMSNorm + RoPE) on 8 TRN2 cores.

Sharding: data-parallel over batch (2) x tensor-parallel over heads (4):
each core handles one batch element, 8 query heads, 2 kv heads, and produces
a partial output (its heads' slice of the Wo contraction); the host sums the
4 partials per batch element.

v2: all matmuls in bf16 (PSUM accumulation stays f32), software-pipelined so
the PE never idles (projection matmuls for block tb+1 are interleaved into
the attention inner loop of block tb), RMSNorm via Sqrt+DVE-reciprocal
(no Ln/Exp table thrash), causal masks applied on GpSimd, V^T produced
directly by the projection (no PE transposes).
"""

import numpy as np
import ml_dtypes

import concourse.bass as bass
import concourse.mybir as mybir
import concourse.tile as tile
from concourse import bacc
from concourse.bass_utils import run_bass_kernel_spmd

# Problem config (hardcoded per contract)
B, T, D = 2, 2048, 2048
H, KV, HD = 32, 8, 64
GROUPS = H // KV
THETA = 10000.0
SCALE = 1.0 / np.sqrt(HD)
EPS = 1e-6

# Per-core sharding
HQL = H // 4          # 8 local q heads
KVL = KV // 4         # 2 local kv heads
FQ = HQL * HD         # 512
FKV = KVL * HD        # 128

# Tiling
P = 128
TB = 512              # token block
NTB = T // TB         # 4
NDC = D // P          # 16 contraction chunks
NKC = T // P          # 16 key chunks
NQC = FQ // P         # 4 q-proj chunks (2 heads each)
NTC = TB // P         # 4 token chunks per block

f32 = mybir.dt.float32
bf16 = mybir.dt.bfloat16
AF = mybir.ActivationFunctionType
ALU = mybir.AluOpType


def _build_nc():
    nc = bacc.Bacc("TRN2", target_bir_lowering=False, debug=False, num_devices=8)

    eps_t = nc.alloc_sbuf_tensor("const-f32-eps", [128, 1], f32)
    nc.gpsimd.memset(eps_t.ap(), EPS)
    nc.const_aps.aps[(f32, EPS)] = eps_t.ap()
    zero_t = nc.alloc_sbuf_tensor("const-f32-zero", [128, 1], f32)
    nc.gpsimd.memset(zero_t.ap(), 0.0)
    nc.const_aps.aps[(f32, 0.0)] = zero_t.ap()
    nc.all_engine_barrier()

    xT_d = nc.dram_tensor("xT", [D, T], bf16, kind="ExternalInput")
    wq_d = nc.dram_tensor("wq", [D, FQ], bf16, kind="ExternalInput")
    wk_d = nc.dram_tensor("wk", [D, FKV], bf16, kind="ExternalInput")
    wv_d = nc.dram_tensor("wv", [D, FKV], bf16, kind="ExternalInput")
    wo_d = nc.dram_tensor("wo", [FQ, D], bf16, kind="ExternalInput")
    cosq_d = nc.dram_tensor("cosq", [P, T], bf16, kind="ExternalInput")
    cosk_d = nc.dram_tensor("cosk", [P, T], bf16, kind="ExternalInput")
    sin_d = nc.dram_tensor("sin", [P, T], bf16, kind="ExternalInput")
    rqT_d = nc.dram_tensor("rqT", [P, P], bf16, kind="ExternalInput")
    rkT_d = nc.dram_tensor("rkT", [P, P], bf16, kind="ExternalInput")
    hsel_d = nc.dram_tensor("hsel", [P, 2], bf16, kind="ExternalInput")
    f32r = mybir.dt.float32r
    hexp_d = nc.dram_tensor("hexp", [2, P], f32r, kind="ExternalInput")
    e1_d = nc.dram_tensor("e1", [1, P], f32r, kind="ExternalInput")
    masks_d = nc.dram_tensor("masks", [P, NTC, TB], bf16, kind="ExternalInput")
    outT_d = nc.dram_tensor("outT", [D, T], f32, kind="ExternalOutput")

    with tile.TileContext(nc) as tc:
        with (
            tc.tile_pool(name="wpool", bufs=1) as wpool,
            tc.tile_pool(name="cpool", bufs=1) as cpool,
            tc.tile_pool(name="kvpool", bufs=1) as kvpool,
            tc.tile_pool(name="xpool", bufs=18) as xpool,
            tc.tile_pool(name="trig", bufs=2) as trig,
            tc.tile_pool(name="bpool", bufs=6) as bpool,
            tc.tile_pool(name="qpool", bufs=2) as qpool,
            tc.tile_pool(name="epool", bufs=6) as epool,
            tc.tile_pool(name="npool", bufs=2) as npool,
            tc.tile_pool(name="opool", bufs=2) as opool,
            tc.tile_pool(name="outp", bufs=3) as outp,
            tc.tile_pool(name="psum", bufs=1, space="PSUM") as psum,
        ):
            # ---- persistent weights / constants ----
            wq_sb = wpool.tile([P, NDC, FQ], bf16)
            wk_sb = wpool.tile([P, NDC, FKV], bf16)
            wv_sb = wpool.tile([P, NDC, FKV], bf16)
            wo_sb = wpool.tile([P, NQC, D], bf16)
            nc.sync.dma_start(wq_sb[:], wq_d.rearrange("(ko p) f -> p ko f", p=P))
            nc.sync.dma_start(wk_sb[:], wk_d.rearrange("(ko p) f -> p ko f", p=P))
            nc.sync.dma_start(wv_sb[:], wv_d.rearrange("(ko p) f -> p ko f", p=P))
            nc.sync.dma_start(wo_sb[:], wo_d.rearrange("(ko p) f -> p ko f", p=P))

            rqT_sb = cpool.tile([P, P], bf16)
            rkT_sb = cpool.tile([P, P], bf16)
            hsel_sb = cpool.tile([P, 2], bf16)
            hexp_sb = cpool.tile([2, P], f32r)
            e1_sb = cpool.tile([1, P], f32r)
            masks_sb = cpool.tile([P, NTC, TB], bf16)
            nc.sync.dma_start(rqT_sb[:], rqT_d[:])
            nc.sync.dma_start(rkT_sb[:], rkT_d[:])
            nc.sync.dma_start(hsel_sb[:], hsel_d[:])
            nc.sync.dma_start(hexp_sb[:], hexp_d[:])
            nc.sync.dma_start(e1_sb[:], e1_d[:])
            nc.sync.dma_start(masks_sb[:], masks_d[:])

            # K^T per-kv-head at both partition placements, V (+ones col)
            ktf = kvpool.tile([P, T], bf16)          # rows 0:64 kv0, 64:128 kv1
            kts = kvpool.tile([P, T], bf16)          # swapped halves
            v_sb = kvpool.tile([P, NKC, KVL, 66], bf16)  # [tok, kc, g, hd+ones+pad]
            ones_bc = nc.const_aps.tensor(1.0, (P, NKC, KVL, 66), bf16)
            nc.vector.tensor_copy(v_sb[:], ones_bc)

            # ---------------------------------------------------------------
            # Feed: projection + square/copy work for token block tbn,
            # returned as a list of closures to be drained into D(tbn-1).
            # ---------------------------------------------------------------
            def make_feed(tbn):
                tbs = slice(tbn * TB, (tbn + 1) * TB)
                st = {}
                ops = []

                def load_xt(dcc):
                    def f():
                        xt = xpool.tile([P, TB], bf16, tag="xt", name=f"xt{dcc}")
                        nc.sync.dma_start(xt[:], xT_d[dcc * P:(dcc + 1) * P, tbs])
                        st[f"xt{dcc}"] = xt
                    return f

                for dcc in range(NDC):
                    ops.append(load_xt(dcc))

                def load_trig():
                    cq_t = trig.tile([P, TB], bf16, tag="cq", name="cq_t")
                    ck_t = trig.tile([P, TB], bf16, tag="ck", name="ck_t")
                    sn_t = trig.tile([P, TB], bf16, tag="sn", name="sn_t")
                    nc.sync.dma_start(cq_t[:], cosq_d[:, tbs])
                    nc.sync.dma_start(ck_t[:], cosk_d[:, tbs])
                    nc.sync.dma_start(sn_t[:], sin_d[:, tbs])
                    st["cq"], st["ck"], st["sn"] = cq_t, ck_t, sn_t

                ops.append(load_trig)

                # q0..q3 and k projection chunks (chunk-major over dc)
                def start_chunk(ci):
                    def f():
                        acc = psum.tile([P, TB], f32, tag="acc", name=f"acc{ci}",
                                        bufs=2)
                        st["acc"] = acc
                    return f

                def mm_chunk(ci, dcc, w_sb, fsl):
                    def f():
                        nc.tensor.matmul(st["acc"][:], w_sb[:, dcc, fsl],
                                         st[f"xt{dcc}"][:],
                                         start=(dcc == 0), stop=(dcc == NDC - 1))
                    return f

                def end_chunk(ci):
                    # square (ACT) + packed sumsq matmul + bf16 copy for RoPE
                    def f():
                        acc = st["acc"]
                        sq = bpool.tile([P, TB], bf16, tag="sq", name=f"sq{ci}")
                        nc.scalar.square(sq[:], acc[:])
                        if ci == 0:
                            ssq = psum.tile([P, TB], f32, tag="ss", name="ssq")
                            st["ssq"] = ssq
                        if ci == 3:
                            ssk = psum.tile([P, TB], f32, tag="sps", name="ssk",
                                            bufs=3)
                            st["ssk"] = ssk
                        # base partition of a PSUM out AP must be 0/32/64:
                        # pack q0-q2 in ssq, q3+k in ssk (sps ring, short-lived)
                        dst = (st["ssq"][32 * ci:32 * ci + 2, :] if ci < 3
                               else st["ssk"][32 * (ci - 3):32 * (ci - 3) + 2, :])
                        nc.tensor.matmul(dst, hsel_sb[:], sq[:],
                                         start=True, stop=True)
                        qsb = bpool.tile([P, TB], bf16, tag="qsb", name=f"qsb{ci}")
                        nc.vector.tensor_copy(qsb[:], acc[:])
                        st[f"qsb{ci}"] = qsb
                    return f

                for ci in range(NQC + 1):  # 4 q chunks then k
                    w_sb = wq_sb if ci < NQC else wk_sb
                    fsl = slice(ci * P, (ci + 1) * P) if ci < NQC else slice(0, FKV)
                    ops.append(start_chunk(ci))
                    for dcc in range(NDC):
                        ops.append(mm_chunk(ci, dcc, w_sb, fsl))
                    ops.append(end_chunk(ci))

                # V^T: out[tok, feat] per 128-token chunk, accumulated over dc
                def start_v():
                    vp = psum.tile([P, TB], f32, tag="acc", name="vpack", bufs=2)
                    st["vpack"] = vp

                ops.append(start_v)

                def mm_v(tcc, dcc):
                    def f():
                        nc.tensor.matmul(
                            st["vpack"][:, tcc * P:(tcc + 1) * P],
                            st[f"xt{dcc}"][:, tcc * P:(tcc + 1) * P],
                            wv_sb[:, dcc, :],
                            start=(dcc == 0), stop=(dcc == NDC - 1))
                    return f

                for tcc in range(NTC):
                    for dcc in range(NDC):
                        ops.append(mm_v(tcc, dcc))

                def end_v():
                    vp = st["vpack"]
                    for tcc in range(NTC):
                        kc = tbn * NTC + tcc
                        nc.vector.tensor_copy(v_sb[:, kc, 0, 0:64],
                                              vp[:, tcc * P:tcc * P + 64])
                        nc.vector.tensor_copy(v_sb[:, kc, 1, 0:64],
                                              vp[:, tcc * P + 64:(tcc + 1) * P])

                ops.append(end_v)
                return st, ops

            # ---------------------------------------------------------------
            # B-rope: grouped rsqrt + RoPE for block tbn (after feed drained)
            # ---------------------------------------------------------------
            qts_cur = {}

            def emit_rope(tbn, st):
                tbs = slice(tbn * TB, (tbn + 1) * TB)
                ssq = st["ssq"]
                # grouped sqrt(ms + eps) on ACT, then reciprocal on DVE
                rrs = []
                for ci in range(NQC + 1):
                    src = (ssq[32 * ci:32 * ci + 2, :] if ci < 3
                           else st["ssk"][32 * (ci - 3):32 * (ci - 3) + 2, :])
                    sst = bpool.tile([2, TB], f32, tag="sst", name=f"sst{ci}", bufs=6)
                    nc.scalar.activation(sst[:], src,
                                         AF.Sqrt, bias=EPS, scale=1.0 / HD)
                    rrs.append(sst)
                rcs = []
                with nc.allow_low_precision(reason="f32r-typed full-f32 recip"):
                    for ci in range(NQC + 1):
                        rr = bpool.tile([2, TB], mybir.dt.float32r, tag="rr",
                                        name=f"rr{ci}", bufs=6)
                        nc.vector.reciprocal(rr[:], rrs[ci][:])
                        rcs.append(rr)

                for ci in range(NQC + 1):
                    is_k = ci == NQC
                    rT = rkT_sb if is_k else rqT_sb
                    ct = st["ck"] if is_k else st["cq"]
                    sn_t = st["sn"]
                    qsb = st[f"qsb{ci}"]
                    bc = psum.tile([P, TB], f32, tag="sps", name=f"bc{ci}", bufs=3)
                    nc.tensor.matmul(bc[:], hexp_sb[:], rcs[ci][:],
                                     start=True, stop=True)
                    qn = bpool.tile([P, TB], bf16, tag="qn", name=f"qn{ci}", bufs=3)
                    nc.vector.tensor_tensor(qn[:], qsb[:], bc[:], ALU.mult)
                    rot = psum.tile([P, TB], f32, tag="sps", name=f"rot{ci}", bufs=3)
                    nc.tensor.matmul(rot[:], rT[:], qn[:], start=True, stop=True)
                    m1 = bpool.tile([P, TB], bf16, tag="m1", name=f"m1_{ci}", bufs=2)
                    nc.vector.tensor_tensor(m1[:], qn[:], ct[:], ALU.mult)
                    m2 = bpool.tile([P, TB], bf16, tag="m2", name=f"m2_{ci}", bufs=2)
                    nc.vector.tensor_tensor(m2[:], rot[:], sn_t[:], ALU.mult)
                    if not is_k:
                        qt = qpool.tile([P, TB], bf16, tag=f"qt{ci}", name=f"qt{ci}")
                        nc.vector.tensor_tensor(qt[:], m1[:], m2[:], ALU.add)
                        qts_cur[ci] = qt
                    else:
                        nc.vector.tensor_tensor(ktf[:, tbs], m1[:], m2[:], ALU.add)
                        nc.vector.tensor_copy(kts[0:64, tbs], ktf[64:P, tbs])
                        nc.vector.tensor_copy(kts[64:P, tbs], ktf[0:64, tbs])

            # ---------------------------------------------------------------
            # D: attention for query block tb, draining `feed` into PE gaps
            # ---------------------------------------------------------------
            def emit_D(tb, feed_ops):
                nkc = (tb + 1) * NTC
                n_iters = KVL * GROUPS * nkc
                fi = 0

                def drain(n):
                    nonlocal fi
                    for _ in range(n):
                        if fi < len(feed_ops):
                            feed_ops[fi]()
                            fi += 1

                per = (len(feed_ops) + n_iters - 1) // n_iters if n_iters else 0

                for g in range(KVL):
                    for pj in range(2):
                        o_pair = []
                        for hh in range(2):
                            hl = GROUPS * g + 2 * pj + hh
                            bq = 64 * (hl % 2)
                            cf = hl // 2
                            kt_tile = ktf if bq == 64 * g else kts
                            o_ps = psum.tile([P, TB], f32, tag="ops",
                                             name=f"ops{hl}", bufs=2)
                            o_pair.append(o_ps)
                            for kc in range(nkc):
                                ksl = slice(kc * P, (kc + 1) * P)
                                sps = psum.tile([P, TB], f32, tag="sps",
                                                name="sps", bufs=3)
                                nc.tensor.matmul(sps[:], kt_tile[bq:bq + 64, ksl],
                                                 qts_cur[cf][bq:bq + 64, :],
                                                 start=True, stop=True)
                                es = epool.tile([P, TB], bf16, tag="es", name="es")
                                nc.scalar.activation(es[:], sps[:], AF.Exp,
                                                     scale=float(SCALE))
                                tdiag = kc - tb * NTC
                                if tdiag >= 0:
                                    # keep es[p,f] where f >= p + 128*tdiag
                                    nc.gpsimd.affine_select(
                                        out=es[:], in_=es[:],
                                        pattern=[[1, TB]],
                                        compare_op=ALU.is_ge, fill=0.0,
                                        base=-P * tdiag,
                                        channel_multiplier=-1)
                                nc.tensor.matmul(o_ps[0:65, :],
                                                 v_sb[:, kc, g, 0:65], es[:],
                                                 start=(kc == 0),
                                                 stop=(kc == nkc - 1))
                                drain(per)
                        # normalize pair -> orhs[cf2]
                        cf2 = 2 * g + pj
                        rpA = npool.tile([1, TB], mybir.dt.float32r, tag="rp",
                                         name="rpA", bufs=4)
                        rpB = npool.tile([1, TB], mybir.dt.float32r, tag="rp",
                                         name="rpB", bufs=4)
                        with nc.allow_low_precision(reason="f32r recip"):
                            nc.vector.reciprocal(rpA[:], o_pair[0][64:65, :])
                            nc.vector.reciprocal(rpB[:], o_pair[1][64:65, :])
                        bc2 = psum.tile([P, TB], f32, tag="sps", name="bc2",
                                        bufs=3)
                        nc.tensor.matmul(bc2[:], hexp_sb[0:1, :], rpA[:],
                                         start=True, stop=False)
                        nc.tensor.matmul(bc2[:], e1_sb[:], rpB[:],
                                         start=False, stop=True)
                        osb = npool.tile([P, TB], bf16, tag="osb", name="osb")
                        nc.vector.tensor_copy(osb[0:64, :], o_pair[0][0:64, :])
                        nc.vector.tensor_copy(osb[64:P, :], o_pair[1][0:64, :])
                        orhs = opool.tile([P, TB], bf16, tag=f"orhs{cf2}",
                                          name=f"orhs{cf2}")
                        nc.vector.tensor_tensor(orhs[:], osb[:], bc2[:], ALU.mult)
                        if cf2 == 0:
                            orhs_l = [None] * NQC
                            st_orhs[0] = orhs_l
                        st_orhs[0][cf2] = orhs
                drain(len(feed_ops))

            st_orhs = [None]

            # ---------------------------------------------------------------
            # E: output projection for block tb
            # ---------------------------------------------------------------
            def emit_E(tb):
                tbs = slice(tb * TB, (tb + 1) * TB)
                orhs_l = st_orhs[0]
                for dc2 in range(NDC):
                    ops_ = psum.tile([P, TB], f32, tag="acc", name="ops_", bufs=2)
                    for cf in range(NQC):
                        nc.tensor.matmul(ops_[:], wo_sb[:, cf, dc2 * P:(dc2 + 1) * P],
                                         orhs_l[cf][:], start=(cf == 0),
                                         stop=(cf == NQC - 1))
                    ob = outp.tile([P, TB], f32, tag="ob", name="ob")
                    if dc2 % 4 == 3:
                        nc.scalar.copy(ob[:], ops_[:])
                    else:
                        nc.vector.tensor_copy(ob[:], ops_[:])
                    nc.sync.dma_start(outT_d[dc2 * P:(dc2 + 1) * P, tbs], ob[:])

            # ---------------------------------------------------------------
            # main schedule
            # ---------------------------------------------------------------
            st0, feed0 = make_feed(0)
            for op in feed0:
                op()
            emit_rope(0, st0)
            for tb in range(NTB):
                if tb + 1 < NTB:
                    st_next, feed_next = make_feed(tb + 1)
                else:
                    st_next, feed_next = None, []
                emit_D(tb, feed_next)
                emit_E(tb)
                if st_next is not None:
                    emit_rope(tb + 1, st_next)

    nc.compile()
    return nc


_NC_CACHE = None


def _get_nc():
    global _NC_CACHE
    if _NC_CACHE is None:
        _NC_CACHE = _build_nc()
    return _NC_CACHE


def _host_constants(q_scale, k_scale):
    pos = np.arange(T, dtype=np.float64)
    invf = 1.0 / (THETA ** (np.arange(0, HD, 2, dtype=np.float64) / HD))  # (32,)
    ang = pos[:, None] * invf[None, :]                                    # (T, 32)
    c = np.cos(ang)
    s = np.sin(ang)
    pidx = np.arange(P) % 32
    hidx = np.arange(P) % HD
    cosq = (c[:, pidx].T * q_scale[hidx][:, None]).astype(ml_dtypes.bfloat16)
    cosk = (c[:, pidx].T * k_scale[hidx][:, None]).astype(ml_dtypes.bfloat16)
    sin = s[:, pidx].T.astype(ml_dtypes.bfloat16)

    def rmat(scale):
        R = np.zeros((HD, HD), dtype=np.float64)
        for i in range(32):
            R[i, i + 32] = -scale[i + 32]
            R[i + 32, i] = scale[i]
        M = np.kron(np.eye(2), R)
        return np.ascontiguousarray(M.T.astype(ml_dtypes.bfloat16))

    hsel = np.zeros((P, 2), dtype=ml_dtypes.bfloat16)
    hsel[0:64, 0] = 1.0
    hsel[64:P, 1] = 1.0
    hexp = np.zeros((2, P), dtype=np.float32)
    hexp[0, 0:64] = 1.0
    hexp[1, 64:P] = 1.0

    masks = np.zeros((P, NTC, TB), dtype=ml_dtypes.bfloat16)
    pp = np.arange(P)[:, None]
    ff = np.arange(TB)[None, :]
    for t in range(NTC):
        masks[:, t, :] = (ff >= pp + P * t).astype(ml_dtypes.bfloat16)

    return cosq, cosk, sin, rmat(q_scale), rmat(k_scale), hsel, hexp, masks


def _run(inputs, trace=False):
    x = np.asarray(inputs["x"], dtype=np.float32)
    Wq = np.asarray(inputs["Wq"], dtype=np.float32)
    Wk = np.asarray(inputs["Wk"], dtype=np.float32)
    Wv = np.asarray(inputs["Wv"], dtype=np.float32)
    Wo = np.asarray(inputs["Wo"], dtype=np.float32)
    q_scale = np.asarray(inputs["q_scale"], dtype=np.float64)
    k_scale = np.asarray(inputs["k_scale"], dtype=np.float64)

    cosq, cosk, sin, rqT, rkT, hsel, hexp, masks = _host_constants(q_scale, k_scale)

    bf = ml_dtypes.bfloat16
    in_maps = []
    for c in range(8):
        b = c // 4
        r = c % 4
        in_maps.append({
            "xT": np.ascontiguousarray(x[b].T).astype(bf),
            "wq": np.ascontiguousarray(Wq[:, r * FQ:(r + 1) * FQ]).astype(bf),
            "wk": np.ascontiguousarray(Wk[:, r * FKV:(r + 1) * FKV]).astype(bf),
            "wv": np.ascontiguousarray(Wv[:, r * FKV:(r + 1) * FKV]).astype(bf),
            "wo": np.ascontiguousarray(Wo[r * FQ:(r + 1) * FQ, :]).astype(bf),
            "cosq": cosq, "cosk": cosk, "sin": sin,
            "rqT": rqT, "rkT": rkT, "hsel": hsel,
            "hexp": hexp, "e1": np.ascontiguousarray(hexp[1:2, :]),
            "masks": masks,
        })

    nc = _get_nc()
    res = run_bass_kernel_spmd(nc, in_maps, core_ids=list(range(8)), trace=trace)
    out = np.empty((B, T, D), dtype=np.float32)
    for b in range(B):
        acc = res.results[4 * b]["outT"].astype(np.float32).copy()
        for r in range(1, 4):
            acc += res.results[4 * b + r]["outT"]
        out[b] = acc.T
    return out, res


def kernel(**inputs):
    out, _ = _run(inputs, trace=False)
    return out


# revision 35
# speedup vs baseline: 1.6010x; 1.1296x over previous
"""GQA (32 q heads / 8 kv heads, head_dim 64, causal, QK-RMSNorm + RoPE) on 8 TRN2 cores.

Sharding: data-parallel over batch (2) x tensor-parallel over heads (4):
each core handles one batch element, 8 query heads, 2 kv heads, and produces
a partial output (its heads' slice of the Wo contraction); the host sums the
4 partials per batch element.

v2: all matmuls in bf16 (PSUM accumulation stays f32), software-pipelined so
the PE never idles (projection matmuls for block tb+1 are interleaved into
the attention inner loop of block tb), RMSNorm via Sqrt+DVE-reciprocal
(no Ln/Exp table thrash), causal masks applied on GpSimd, V^T produced
directly by the projection (no PE transposes).
"""

import numpy as np
import ml_dtypes

import concourse.bass as bass
import concourse.mybir as mybir
import concourse.tile as tile
from concourse import bacc
from concourse.bass_utils import run_bass_kernel_spmd

# Problem config (hardcoded per contract)
B, T, D = 2, 2048, 2048
H, KV, HD = 32, 8, 64
GROUPS = H // KV
THETA = 10000.0
SCALE = 1.0 / np.sqrt(HD)
EPS = 1e-6

# Per-core sharding
HQL = H // 4          # 8 local q heads
KVL = KV // 4         # 2 local kv heads
FQ = HQL * HD         # 512
FKV = KVL * HD        # 128

# Tiling
P = 128
TB = 512              # token block
NTB = T // TB         # 4
NDC = D // P          # 16 contraction chunks
NKC = T // P          # 16 key chunks
NQC = FQ // P         # 4 q-proj chunks (2 heads each)
NTC = TB // P         # 4 token chunks per block

f32 = mybir.dt.float32
bf16 = mybir.dt.bfloat16
AF = mybir.ActivationFunctionType
ALU = mybir.AluOpType


def _build_nc():
    nc = bacc.Bacc("TRN2", target_bir_lowering=False, debug=False, num_devices=8)

    eps_t = nc.alloc_sbuf_tensor("const-f32-eps", [128, 1], f32)
    nc.gpsimd.memset(eps_t.ap(), EPS)
    nc.const_aps.aps[(f32, EPS)] = eps_t.ap()
    zero_t = nc.alloc_sbuf_tensor("const-f32-zero", [128, 1], f32)
    nc.gpsimd.memset(zero_t.ap(), 0.0)
    nc.const_aps.aps[(f32, 0.0)] = zero_t.ap()
    nc.all_engine_barrier()

    xT_d = nc.dram_tensor("xT", [D, T], bf16, kind="ExternalInput")
    wq_d = nc.dram_tensor("wq", [D, FQ], bf16, kind="ExternalInput")
    wk_d = nc.dram_tensor("wk", [D, FKV], bf16, kind="ExternalInput")
    wv_d = nc.dram_tensor("wv", [D, FKV], bf16, kind="ExternalInput")
    wo_d = nc.dram_tensor("wo", [FQ, D], bf16, kind="ExternalInput")
    cosq_d = nc.dram_tensor("cosq", [P, T], bf16, kind="ExternalInput")
    cosk_d = nc.dram_tensor("cosk", [P, T], bf16, kind="ExternalInput")
    sin_d = nc.dram_tensor("sin", [P, T], bf16, kind="ExternalInput")
    rqT_d = nc.dram_tensor("rqT", [P, P], bf16, kind="ExternalInput")
    rkT_d = nc.dram_tensor("rkT", [P, P], bf16, kind="ExternalInput")
    hsel_d = nc.dram_tensor("hsel", [P, 2], bf16, kind="ExternalInput")
    hexp_d = nc.dram_tensor("hexp", [2, P], bf16, kind="ExternalInput")
    e1_d = nc.dram_tensor("e1", [1, P], bf16, kind="ExternalInput")
    masks_d = nc.dram_tensor("masks", [P, NTC, TB], bf16, kind="ExternalInput")
    outT_d = nc.dram_tensor("outT", [D, T], f32, kind="ExternalOutput")

    with tile.TileContext(nc) as tc:
        with (
            tc.tile_pool(name="wpool", bufs=1) as wpool,
            tc.tile_pool(name="cpool", bufs=1) as cpool,
            tc.tile_pool(name="kvpool", bufs=1) as kvpool,
            tc.tile_pool(name="xpool", bufs=18) as xpool,
            tc.tile_pool(name="trig", bufs=2) as trig,
            tc.tile_pool(name="bpool", bufs=6) as bpool,
            tc.tile_pool(name="qpool", bufs=2) as qpool,
            tc.tile_pool(name="epool", bufs=6) as epool,
            tc.tile_pool(name="npool", bufs=2) as npool,
            tc.tile_pool(name="opool", bufs=2) as opool,
            tc.tile_pool(name="outp", bufs=3) as outp,
            tc.tile_pool(name="psum", bufs=1, space="PSUM") as psum,
        ):
            # ---- persistent weights / constants ----
            wq_sb = wpool.tile([P, NDC, FQ], bf16)
            wk_sb = wpool.tile([P, NDC, FKV], bf16)
            wv_sb = wpool.tile([P, NDC, FKV], bf16)
            wo_sb = wpool.tile([P, NQC, D], bf16)
            nc.sync.dma_start(wq_sb[:], wq_d.rearrange("(ko p) f -> p ko f", p=P))
            nc.sync.dma_start(wk_sb[:], wk_d.rearrange("(ko p) f -> p ko f", p=P))
            nc.sync.dma_start(wv_sb[:], wv_d.rearrange("(ko p) f -> p ko f", p=P))
            nc.sync.dma_start(wo_sb[:], wo_d.rearrange("(ko p) f -> p ko f", p=P))

            rqT_sb = cpool.tile([P, P], bf16)
            rkT_sb = cpool.tile([P, P], bf16)
            hsel_sb = cpool.tile([P, 2], bf16)
            hexp_sb = cpool.tile([2, P], bf16)
            e1_sb = cpool.tile([1, P], bf16)
            masks_sb = cpool.tile([P, NTC, TB], bf16)
            nc.sync.dma_start(rqT_sb[:], rqT_d[:])
            nc.sync.dma_start(rkT_sb[:], rkT_d[:])
            nc.sync.dma_start(hsel_sb[:], hsel_d[:])
            nc.sync.dma_start(hexp_sb[:], hexp_d[:])
            nc.sync.dma_start(e1_sb[:], e1_d[:])
            nc.sync.dma_start(masks_sb[:], masks_d[:])

            # K^T per-kv-head at both partition placements, V (+ones col)
            ktf = kvpool.tile([P, T], bf16)          # rows 0:64 kv0, 64:128 kv1
            kts = kvpool.tile([P, T], bf16)          # swapped halves
            v_sb = kvpool.tile([P, NKC, KVL, 66], bf16)  # [tok, kc, g, hd+ones+pad]
            ones_bc = nc.const_aps.tensor(1.0, (P, NKC, KVL, 66), bf16)
            nc.vector.tensor_copy(v_sb[:], ones_bc)

            # ---------------------------------------------------------------
            # Feed: projection + square/copy work for token block tbn,
            # returned as a list of closures to be drained into D(tbn-1).
            # ---------------------------------------------------------------
            def make_feed(tbn):
                tbs = slice(tbn * TB, (tbn + 1) * TB)
                st = {}
                ops = []

                def load_xt(dcc):
                    def f():
                        xt = xpool.tile([P, TB], bf16, tag="xt", name=f"xt{dcc}")
                        nc.sync.dma_start(xt[:], xT_d[dcc * P:(dcc + 1) * P, tbs])
                        st[f"xt{dcc}"] = xt
                    return f

                for dcc in range(NDC):
                    ops.append(load_xt(dcc))

                def load_trig():
                    cq_t = trig.tile([P, TB], bf16, tag="cq", name="cq_t")
                    ck_t = trig.tile([P, TB], bf16, tag="ck", name="ck_t")
                    sn_t = trig.tile([P, TB], bf16, tag="sn", name="sn_t")
                    nc.sync.dma_start(cq_t[:], cosq_d[:, tbs])
                    nc.sync.dma_start(ck_t[:], cosk_d[:, tbs])
                    nc.sync.dma_start(sn_t[:], sin_d[:, tbs])
                    st["cq"], st["ck"], st["sn"] = cq_t, ck_t, sn_t

                ops.append(load_trig)

                # q0..q3 and k projection chunks (chunk-major over dc)
                def start_chunk(ci):
                    def f():
                        acc = psum.tile([P, TB], f32, tag="acc", name=f"acc{ci}",
                                        bufs=2)
                        st["acc"] = acc
                    return f

                def mm_chunk(ci, dcc, w_sb, fsl):
                    def f():
                        nc.tensor.matmul(st["acc"][:], w_sb[:, dcc, fsl],
                                         st[f"xt{dcc}"][:],
                                         start=(dcc == 0), stop=(dcc == NDC - 1))
                    return f

                def end_chunk(ci):
                    # bf16 copy for RoPE, square on DVE, packed sumsq matmul
                    def f():
                        acc = st["acc"]
                        qsb = bpool.tile([P, TB], bf16, tag="qsb", name=f"qsb{ci}")
                        nc.vector.tensor_copy(qsb[:], acc[:])
                        st[f"qsb{ci}"] = qsb
                        sq = bpool.tile([P, TB], bf16, tag="sq", name=f"sq{ci}")
                        nc.vector.tensor_tensor(sq[:], qsb[:], qsb[:], ALU.mult)
                        if ci == 0:
                            ssq = psum.tile([P, TB], f32, tag="ss", name="ssq")
                            st["ssq"] = ssq
                        if ci == 3:
                            ssk = psum.tile([P, TB], f32, tag="sps", name="ssk",
                                            bufs=3)
                            st["ssk"] = ssk
                        # base partition of a PSUM out AP must be 0/32/64:
                        # pack q0-q2 in ssq, q3+k in ssk (sps ring, short-lived)
                        dst = (st["ssq"][32 * ci:32 * ci + 2, :] if ci < 3
                               else st["ssk"][32 * (ci - 3):32 * (ci - 3) + 2, :])
                        nc.tensor.matmul(dst, hsel_sb[:], sq[:],
                                         start=True, stop=True)
                    return f

                for ci in range(NQC + 1):  # 4 q chunks then k
                    w_sb = wq_sb if ci < NQC else wk_sb
                    fsl = slice(ci * P, (ci + 1) * P) if ci < NQC else slice(0, FKV)
                    ops.append(start_chunk(ci))
                    for dcc in range(NDC):
                        ops.append(mm_chunk(ci, dcc, w_sb, fsl))
                    ops.append(end_chunk(ci))

                # V^T: out[tok, feat] per 128-token chunk, accumulated over dc
                def start_v():
                    vp = psum.tile([P, TB], f32, tag="acc", name="vpack", bufs=2)
                    st["vpack"] = vp

                ops.append(start_v)

                def mm_v(tcc, dcc):
                    def f():
                        nc.tensor.matmul(
                            st["vpack"][:, tcc * P:(tcc + 1) * P],
                            st[f"xt{dcc}"][:, tcc * P:(tcc + 1) * P],
                            wv_sb[:, dcc, :],
                            start=(dcc == 0), stop=(dcc == NDC - 1))
                    return f

                for tcc in range(NTC):
                    for dcc in range(NDC):
                        ops.append(mm_v(tcc, dcc))

                def end_v():
                    vp = st["vpack"]
                    for tcc in range(NTC):
                        kc = tbn * NTC + tcc
                        nc.vector.tensor_copy(v_sb[:, kc, 0, 0:64],
                                              vp[:, tcc * P:tcc * P + 64])
                        nc.vector.tensor_copy(v_sb[:, kc, 1, 0:64],
                                              vp[:, tcc * P + 64:(tcc + 1) * P])

                ops.append(end_v)
                return st, ops

            # ---------------------------------------------------------------
            # B-rope: grouped rsqrt + RoPE for block tbn (after feed drained)
            # ---------------------------------------------------------------
            qts_cur = {}

            def emit_rope(tbn, st):
                tbs = slice(tbn * TB, (tbn + 1) * TB)
                ssq = st["ssq"]
                # grouped sqrt(ms + eps) on ACT, then reciprocal on DVE
                rrs = []
                for ci in range(NQC + 1):
                    src = (ssq[32 * ci:32 * ci + 2, :] if ci < 3
                           else st["ssk"][32 * (ci - 3):32 * (ci - 3) + 2, :])
                    sst = bpool.tile([2, TB], f32, tag="sst", name=f"sst{ci}", bufs=6)
                    nc.scalar.activation(sst[:], src,
                                         AF.Sqrt, bias=EPS, scale=1.0 / HD)
                    rrs.append(sst)
                rcs = []
                for ci in range(NQC + 1):
                    rr = bpool.tile([2, TB], f32, tag="rr", name=f"rr{ci}", bufs=6)
                    nc.vector.reciprocal_approx_fast(rr[:], rrs[ci][:])
                    rrb = bpool.tile([2, TB], bf16, tag="rrb", name=f"rrb{ci}",
                                     bufs=6)
                    nc.vector.tensor_copy(rrb[:], rr[:])
                    rcs.append(rrb)

                for ci in range(NQC + 1):
                    is_k = ci == NQC
                    rT = rkT_sb if is_k else rqT_sb
                    ct = st["ck"] if is_k else st["cq"]
                    sn_t = st["sn"]
                    qsb = st[f"qsb{ci}"]
                    bc = psum.tile([P, TB], f32, tag="sps", name=f"bc{ci}", bufs=3)
                    nc.tensor.matmul(bc[:], hexp_sb[:], rcs[ci][:],
                                     start=True, stop=True)
                    qn = bpool.tile([P, TB], bf16, tag="qn", name=f"qn{ci}", bufs=3)
                    nc.vector.tensor_tensor(qn[:], qsb[:], bc[:], ALU.mult)
                    rot = psum.tile([P, TB], f32, tag="sps", name=f"rot{ci}", bufs=3)
                    nc.tensor.matmul(rot[:], rT[:], qn[:], start=True, stop=True)
                    m1 = bpool.tile([P, TB], bf16, tag="m1", name=f"m1_{ci}", bufs=2)
                    nc.vector.tensor_tensor(m1[:], qn[:], ct[:], ALU.mult)
                    m2 = bpool.tile([P, TB], bf16, tag="m2", name=f"m2_{ci}", bufs=2)
                    nc.vector.tensor_tensor(m2[:], rot[:], sn_t[:], ALU.mult)
                    if not is_k:
                        qt = qpool.tile([P, TB], bf16, tag=f"qt{ci}", name=f"qt{ci}")
                        nc.vector.tensor_tensor(qt[:], m1[:], m2[:], ALU.add)
                        qts_cur[ci] = qt
                    else:
                        nc.vector.tensor_tensor(ktf[:, tbs], m1[:], m2[:], ALU.add)
                        nc.vector.tensor_copy(kts[0:64, tbs], ktf[64:P, tbs])
                        nc.vector.tensor_copy(kts[64:P, tbs], ktf[0:64, tbs])

            # ---------------------------------------------------------------
            # D: attention for query block tb, draining `feed` into PE gaps
            # ---------------------------------------------------------------
            def emit_D(tb, feed_ops, rope_hook=None):
                nkc = (tb + 1) * NTC
                n_iters = KVL * GROUPS * nkc
                qts = dict(qts_cur)   # rope_hook rebinds qts_cur for tb+1
                fi = 0

                def drain(n):
                    nonlocal fi
                    for _ in range(n):
                        if fi < len(feed_ops):
                            feed_ops[fi]()
                            fi += 1

                # front-load the feed so it completes ~60% through D
                per = ((len(feed_ops) + int(n_iters * 0.6)) // max(1, int(n_iters * 0.6))
                       if n_iters else 0)

                for g in range(KVL):
                    for pj in range(2):
                        if g == 1 and pj == 1:
                            drain(len(feed_ops))
                            if rope_hook is not None:
                                rope_hook()
                        o_pair = []
                        for hh in range(2):
                            hl = GROUPS * g + 2 * pj + hh
                            bq = 64 * (hl % 2)
                            cf = hl // 2
                            kt_tile = ktf if bq == 64 * g else kts
                            o_ps = psum.tile([P, TB], f32, tag="ops",
                                             name=f"ops{hl}", bufs=2)
                            o_pair.append(o_ps)
                            # off-diagonal blocks: exp straight to AV
                            for kc in range(tb * NTC):
                                ksl = slice(kc * P, (kc + 1) * P)
                                sps = psum.tile([P, TB], f32, tag="sps",
                                                name="sps", bufs=3)
                                nc.tensor.matmul(sps[:], kt_tile[bq:bq + 64, ksl],
                                                 qts[cf][bq:bq + 64, :],
                                                 start=True, stop=True)
                                es = epool.tile([P, TB], bf16, tag="es", name="es")
                                nc.scalar.activation(es[:], sps[:], AF.Exp,
                                                     scale=float(SCALE))
                                nc.tensor.matmul(o_ps[0:65, :],
                                                 v_sb[:, kc, g, 0:65], es[:],
                                                 start=(kc == 0),
                                                 stop=False)
                                drain(per)
                            # diagonal band: batch 4 chunks of exp into esd,
                            # one affine_select, then the 4 deferred AVs
                            esd = epool.tile([P, NTC, TB], bf16, tag="esd",
                                             name="esd", bufs=2)
                            for tdiag in range(NTC):
                                kc = tb * NTC + tdiag
                                ksl = slice(kc * P, (kc + 1) * P)
                                sps = psum.tile([P, TB], f32, tag="sps",
                                                name="sps", bufs=3)
                                nc.tensor.matmul(sps[:], kt_tile[bq:bq + 64, ksl],
                                                 qts[cf][bq:bq + 64, :],
                                                 start=True, stop=True)
                                nc.scalar.activation(esd[:, tdiag, :], sps[:],
                                                     AF.Exp, scale=float(SCALE))
                                drain(per)
                            # keep esd[p, t, f] where f >= p + 128*t
                            nc.gpsimd.affine_select(
                                out=esd[:], in_=esd[:],
                                pattern=[[-P, NTC], [1, TB]],
                                compare_op=ALU.is_ge, fill=0.0,
                                base=0, channel_multiplier=-1)
                            for tdiag in range(NTC):
                                kc = tb * NTC + tdiag
                                nc.tensor.matmul(o_ps[0:65, :],
                                                 v_sb[:, kc, g, 0:65],
                                                 esd[:, tdiag, :],
                                                 start=(kc == 0),
                                                 stop=(kc == nkc - 1))
                        # normalize pair -> orhs[cf2]
                        cf2 = 2 * g + pj
                        dnA = npool.tile([1, TB], f32, tag="dn", name="dnA", bufs=4)
                        dnB = npool.tile([1, TB], f32, tag="dn", name="dnB", bufs=4)
                        nc.vector.tensor_copy(dnA[:], o_pair[0][64:65, :])
                        nc.vector.tensor_copy(dnB[:], o_pair[1][64:65, :])
                        rpA = npool.tile([1, TB], f32, tag="rp", name="rpA", bufs=4)
                        rpB = npool.tile([1, TB], f32, tag="rp", name="rpB", bufs=4)
                        nc.vector.reciprocal_approx_fast(rpA[:], dnA[:])
                        nc.vector.reciprocal_approx_fast(rpB[:], dnB[:])
                        rpAb = npool.tile([1, TB], bf16, tag="rpb", name="rpAb",
                                          bufs=4)
                        rpBb = npool.tile([1, TB], bf16, tag="rpb", name="rpBb",
                                          bufs=4)
                        nc.vector.tensor_copy(rpAb[:], rpA[:])
                        nc.vector.tensor_copy(rpBb[:], rpB[:])
                        bc2 = psum.tile([P, TB], f32, tag="sps", name="bc2",
                                        bufs=3)
                        nc.tensor.matmul(bc2[:], hexp_sb[0:1, :], rpAb[:],
                                         start=True, stop=False)
                        nc.tensor.matmul(bc2[:], e1_sb[:], rpBb[:],
                                         start=False, stop=True)
                        osb = npool.tile([P, TB], bf16, tag="osb", name="osb")
                        nc.vector.tensor_copy(osb[0:64, :], o_pair[0][0:64, :])
                        nc.vector.tensor_copy(osb[64:P, :], o_pair[1][0:64, :])
                        orhs = opool.tile([P, TB], bf16, tag=f"orhs{cf2}",
                                          name=f"orhs{cf2}")
                        nc.vector.tensor_tensor(orhs[:], osb[:], bc2[:], ALU.mult)
                        if cf2 == 0:
                            orhs_l = [None] * NQC
                            st_orhs[0] = orhs_l
                        st_orhs[0][cf2] = orhs
                drain(len(feed_ops))

            st_orhs = [None]

            # ---------------------------------------------------------------
            # E: output projection for block tb
            # ---------------------------------------------------------------
            def emit_E(tb):
                tbs = slice(tb * TB, (tb + 1) * TB)
                orhs_l = st_orhs[0]
                for dc2 in range(NDC):
                    ops_ = psum.tile([P, TB], f32, tag="acc", name="ops_", bufs=2)
                    for cf in range(NQC):
                        nc.tensor.matmul(ops_[:], wo_sb[:, cf, dc2 * P:(dc2 + 1) * P],
                                         orhs_l[cf][:], start=(cf == 0),
                                         stop=(cf == NQC - 1))
                    ob = outp.tile([P, TB], f32, tag="ob", name="ob")
                    if dc2 % 4 == 3:
                        nc.scalar.copy(ob[:], ops_[:])
                    else:
                        nc.vector.tensor_copy(ob[:], ops_[:])
                    nc.sync.dma_start(outT_d[dc2 * P:(dc2 + 1) * P, tbs], ob[:])

            # ---------------------------------------------------------------
            # main schedule
            # ---------------------------------------------------------------
            st0, feed0 = make_feed(0)
            for op in feed0:
                op()
            emit_rope(0, st0)
            for tb in range(NTB):
                if tb + 1 < NTB:
                    st_next, feed_next = make_feed(tb + 1)
                    hook = (lambda s=st_next, t=tb + 1: emit_rope(t, s))
                else:
                    st_next, feed_next, hook = None, [], None
                emit_D(tb, feed_next, rope_hook=hook)
                emit_E(tb)

    nc.compile()
    return nc


_NC_CACHE = None


def _get_nc():
    global _NC_CACHE
    if _NC_CACHE is None:
        _NC_CACHE = _build_nc()
    return _NC_CACHE


def _host_constants(q_scale, k_scale):
    pos = np.arange(T, dtype=np.float64)
    invf = 1.0 / (THETA ** (np.arange(0, HD, 2, dtype=np.float64) / HD))  # (32,)
    ang = pos[:, None] * invf[None, :]                                    # (T, 32)
    c = np.cos(ang)
    s = np.sin(ang)
    pidx = np.arange(P) % 32
    hidx = np.arange(P) % HD
    cosq = (c[:, pidx].T * q_scale[hidx][:, None]).astype(ml_dtypes.bfloat16)
    cosk = (c[:, pidx].T * k_scale[hidx][:, None]).astype(ml_dtypes.bfloat16)
    sin = s[:, pidx].T.astype(ml_dtypes.bfloat16)

    def rmat(scale):
        R = np.zeros((HD, HD), dtype=np.float64)
        for i in range(32):
            R[i, i + 32] = -scale[i + 32]
            R[i + 32, i] = scale[i]
        M = np.kron(np.eye(2), R)
        return np.ascontiguousarray(M.T.astype(ml_dtypes.bfloat16))

    hsel = np.zeros((P, 2), dtype=ml_dtypes.bfloat16)
    hsel[0:64, 0] = 1.0
    hsel[64:P, 1] = 1.0
    hexp = np.zeros((2, P), dtype=ml_dtypes.bfloat16)
    hexp[0, 0:64] = 1.0
    hexp[1, 64:P] = 1.0

    masks = np.zeros((P, NTC, TB), dtype=ml_dtypes.bfloat16)
    pp = np.arange(P)[:, None]
    ff = np.arange(TB)[None, :]
    for t in range(NTC):
        masks[:, t, :] = (ff >= pp + P * t).astype(ml_dtypes.bfloat16)

    return cosq, cosk, sin, rmat(q_scale), rmat(k_scale), hsel, hexp, masks


def _run(inputs, trace=False):
    x = np.asarray(inputs["x"], dtype=np.float32)
    Wq = np.asarray(inputs["Wq"], dtype=np.float32)
    Wk = np.asarray(inputs["Wk"], dtype=np.float32)
    Wv = np.asarray(inputs["Wv"], dtype=np.float32)
    Wo = np.asarray(inputs["Wo"], dtype=np.float32)
    q_scale = np.asarray(inputs["q_scale"], dtype=np.float64)
    k_scale = np.asarray(inputs["k_scale"], dtype=np.float64)

    cosq, cosk, sin, rqT, rkT, hsel, hexp, masks = _host_constants(q_scale, k_scale)

    bf = ml_dtypes.bfloat16
    in_maps = []
    for c in range(8):
        b = c // 4
        r = c % 4
        in_maps.append({
            "xT": np.ascontiguousarray(x[b].T).astype(bf),
            "wq": np.ascontiguousarray(Wq[:, r * FQ:(r + 1) * FQ]).astype(bf),
            "wk": np.ascontiguousarray(Wk[:, r * FKV:(r + 1) * FKV]).astype(bf),
            "wv": np.ascontiguousarray(Wv[:, r * FKV:(r + 1) * FKV]).astype(bf),
            "wo": np.ascontiguousarray(Wo[r * FQ:(r + 1) * FQ, :]).astype(bf),
            "cosq": cosq, "cosk": cosk, "sin": sin,
            "rqT": rqT, "rkT": rkT, "hsel": hsel,
            "hexp": hexp, "e1": np.ascontiguousarray(hexp[1:2, :]),
            "masks": masks,
        })

    nc = _get_nc()
    res = run_bass_kernel_spmd(nc, in_maps, core_ids=list(range(8)), trace=trace)
    out = np.empty((B, T, D), dtype=np.float32)
    for b in range(B):
        acc = res.results[4 * b]["outT"].astype(np.float32).copy()
        for r in range(1, 4):
            acc += res.results[4 * b + r]["outT"]
        out[b] = acc.T
    return out, res


def kernel(**inputs):
    out, _ = _run(inputs, trace=False)
    return out


# revision 38
# speedup vs baseline: 1.6165x; 1.0097x over previous
"""GQA (32 q heads / 8 kv heads, head_dim 64, causal, QK-RMSNorm + RoPE) on 8 TRN2 cores.

Sharding: data-parallel over batch (2) x tensor-parallel over heads (4):
each core handles one batch element, 8 query heads, 2 kv heads, and produces
a partial output (its heads' slice of the Wo contraction); the host sums the
4 partials per batch element.

v2: all matmuls in bf16 (PSUM accumulation stays f32), software-pipelined so
the PE never idles (projection matmuls for block tb+1 are interleaved into
the attention inner loop of block tb), RMSNorm via Sqrt+DVE-reciprocal
(no Ln/Exp table thrash), causal masks applied on GpSimd, V^T produced
directly by the projection (no PE transposes).
"""

import numpy as np
import ml_dtypes

import concourse.bass as bass
import concourse.mybir as mybir
import concourse.tile as tile
from concourse import bacc
from concourse.bass_utils import run_bass_kernel_spmd

# Problem config (hardcoded per contract)
B, T, D = 2, 2048, 2048
H, KV, HD = 32, 8, 64
GROUPS = H // KV
THETA = 10000.0
SCALE = 1.0 / np.sqrt(HD)
EPS = 1e-6

# Per-core sharding
HQL = H // 4          # 8 local q heads
KVL = KV // 4         # 2 local kv heads
FQ = HQL * HD         # 512
FKV = KVL * HD        # 128

# Tiling
P = 128
TB = 512              # token block
NTB = T // TB         # 4
NDC = D // P          # 16 contraction chunks
NKC = T // P          # 16 key chunks
NQC = FQ // P         # 4 q-proj chunks (2 heads each)
NTC = TB // P         # 4 token chunks per block

f32 = mybir.dt.float32
bf16 = mybir.dt.bfloat16
AF = mybir.ActivationFunctionType
ALU = mybir.AluOpType


def _build_nc():
    nc = bacc.Bacc("TRN2", target_bir_lowering=False, debug=False, num_devices=8)

    eps_t = nc.alloc_sbuf_tensor("const-f32-eps", [128, 1], f32)
    nc.gpsimd.memset(eps_t.ap(), EPS)
    nc.const_aps.aps[(f32, EPS)] = eps_t.ap()
    zero_t = nc.alloc_sbuf_tensor("const-f32-zero", [128, 1], f32)
    nc.gpsimd.memset(zero_t.ap(), 0.0)
    nc.const_aps.aps[(f32, 0.0)] = zero_t.ap()
    nc.all_engine_barrier()

    xT_d = nc.dram_tensor("xT", [D, T], bf16, kind="ExternalInput")
    wq_d = nc.dram_tensor("wq", [D, FQ], bf16, kind="ExternalInput")
    wk_d = nc.dram_tensor("wk", [D, FKV], bf16, kind="ExternalInput")
    wv_d = nc.dram_tensor("wv", [D, FKV], bf16, kind="ExternalInput")
    wo_d = nc.dram_tensor("wo", [FQ, D], bf16, kind="ExternalInput")
    cosq_d = nc.dram_tensor("cosq", [P, T], bf16, kind="ExternalInput")
    cosk_d = nc.dram_tensor("cosk", [P, T], bf16, kind="ExternalInput")
    sin_d = nc.dram_tensor("sin", [P, T], bf16, kind="ExternalInput")
    rqT_d = nc.dram_tensor("rqT", [P, P], bf16, kind="ExternalInput")
    rkT_d = nc.dram_tensor("rkT", [P, P], bf16, kind="ExternalInput")
    hsel_d = nc.dram_tensor("hsel", [P, 2], bf16, kind="ExternalInput")
    hexp_d = nc.dram_tensor("hexp", [2, P], bf16, kind="ExternalInput")
    e1_d = nc.dram_tensor("e1", [1, P], bf16, kind="ExternalInput")
    masks_d = nc.dram_tensor("masks", [P, NTC, TB], bf16, kind="ExternalInput")
    outT_d = nc.dram_tensor("outT", [D, T], f32, kind="ExternalOutput")

    with tile.TileContext(nc) as tc:
        with (
            tc.tile_pool(name="wpool", bufs=1) as wpool,
            tc.tile_pool(name="cpool", bufs=1) as cpool,
            tc.tile_pool(name="kvpool", bufs=1) as kvpool,
            tc.tile_pool(name="xpool", bufs=18) as xpool,
            tc.tile_pool(name="trig", bufs=2) as trig,
            tc.tile_pool(name="bpool", bufs=6) as bpool,
            tc.tile_pool(name="qpool", bufs=2) as qpool,
            tc.tile_pool(name="epool", bufs=6) as epool,
            tc.tile_pool(name="npool", bufs=2) as npool,
            tc.tile_pool(name="opool", bufs=2) as opool,
            tc.tile_pool(name="outp", bufs=3) as outp,
            tc.tile_pool(name="psum", bufs=1, space="PSUM") as psum,
        ):
            # ---- persistent weights / constants ----
            wq_sb = wpool.tile([P, NDC, FQ], bf16)
            wk_sb = wpool.tile([P, NDC, FKV], bf16)
            wv_sb = wpool.tile([P, NDC, FKV], bf16)
            wo_sb = wpool.tile([P, NQC, D], bf16)
            nc.sync.dma_start(wq_sb[:], wq_d.rearrange("(ko p) f -> p ko f", p=P))
            nc.sync.dma_start(wk_sb[:], wk_d.rearrange("(ko p) f -> p ko f", p=P))
            nc.sync.dma_start(wv_sb[:], wv_d.rearrange("(ko p) f -> p ko f", p=P))
            nc.sync.dma_start(wo_sb[:], wo_d.rearrange("(ko p) f -> p ko f", p=P))

            rqT_sb = cpool.tile([P, P], bf16)
            rkT_sb = cpool.tile([P, P], bf16)
            hsel_sb = cpool.tile([P, 2], bf16)
            hexp_sb = cpool.tile([2, P], bf16)
            e1_sb = cpool.tile([1, P], bf16)
            masks_sb = cpool.tile([P, NTC, TB], bf16)
            nc.sync.dma_start(rqT_sb[:], rqT_d[:])
            nc.sync.dma_start(rkT_sb[:], rkT_d[:])
            nc.sync.dma_start(hsel_sb[:], hsel_d[:])
            nc.sync.dma_start(hexp_sb[:], hexp_d[:])
            nc.sync.dma_start(e1_sb[:], e1_d[:])
            nc.sync.dma_start(masks_sb[:], masks_d[:])

            # K^T per-kv-head at both partition placements, V (+ones col)
            ktf = kvpool.tile([P, T], bf16)          # rows 0:64 kv0, 64:128 kv1
            kts = kvpool.tile([P, T], bf16)          # swapped halves
            v_sb = kvpool.tile([P, NKC, KVL, 66], bf16)  # [tok, kc, g, hd+ones+pad]
            ones_bc = nc.const_aps.tensor(1.0, (P, NKC, KVL, 66), bf16)
            nc.vector.tensor_copy(v_sb[:], ones_bc)

            # ---------------------------------------------------------------
            # Feed: projection + square/copy work for token block tbn,
            # returned as a list of closures to be drained into D(tbn-1).
            # ---------------------------------------------------------------
            def make_feed(tbn):
                tbs = slice(tbn * TB, (tbn + 1) * TB)
                st = {}
                ops = []

                def load_xt(dcc):
                    def f():
                        xt = xpool.tile([P, TB], bf16, tag="xt", name=f"xt{dcc}")
                        nc.sync.dma_start(xt[:], xT_d[dcc * P:(dcc + 1) * P, tbs])
                        st[f"xt{dcc}"] = xt
                    return f

                for dcc in range(NDC):
                    ops.append(load_xt(dcc))

                def load_trig():
                    cq_t = trig.tile([P, TB], bf16, tag="cq", name="cq_t")
                    ck_t = trig.tile([P, TB], bf16, tag="ck", name="ck_t")
                    sn_t = trig.tile([P, TB], bf16, tag="sn", name="sn_t")
                    nc.sync.dma_start(cq_t[:], cosq_d[:, tbs])
                    nc.sync.dma_start(ck_t[:], cosk_d[:, tbs])
                    nc.sync.dma_start(sn_t[:], sin_d[:, tbs])
                    st["cq"], st["ck"], st["sn"] = cq_t, ck_t, sn_t

                ops.append(load_trig)

                # q0..q3 and k projection chunks (chunk-major over dc)
                def start_chunk(ci):
                    def f():
                        acc = psum.tile([P, TB], f32, tag="acc", name=f"acc{ci}",
                                        bufs=2)
                        st["acc"] = acc
                    return f

                def mm_chunk(ci, dcc, w_sb, fsl):
                    def f():
                        nc.tensor.matmul(st["acc"][:], w_sb[:, dcc, fsl],
                                         st[f"xt{dcc}"][:],
                                         start=(dcc == 0), stop=(dcc == NDC - 1))
                    return f

                def end_chunk(ci):
                    # bf16 copy for RoPE, square on DVE, packed sumsq matmul
                    def f():
                        acc = st["acc"]
                        qsb = bpool.tile([P, TB], bf16, tag="qsb", name=f"qsb{ci}")
                        nc.vector.tensor_copy(qsb[:], acc[:])
                        st[f"qsb{ci}"] = qsb
                        sq = bpool.tile([P, TB], bf16, tag="sq", name=f"sq{ci}")
                        nc.vector.tensor_tensor(sq[:], qsb[:], qsb[:], ALU.mult)
                        if ci == 0:
                            ssq = psum.tile([P, TB], f32, tag="ss", name="ssq")
                            st["ssq"] = ssq
                        if ci == 3:
                            ssk = psum.tile([P, TB], f32, tag="sps", name="ssk",
                                            bufs=3)
                            st["ssk"] = ssk
                        # base partition of a PSUM out AP must be 0/32/64:
                        # pack q0-q2 in ssq, q3+k in ssk (sps ring, short-lived)
                        dst = (st["ssq"][32 * ci:32 * ci + 2, :] if ci < 3
                               else st["ssk"][32 * (ci - 3):32 * (ci - 3) + 2, :])
                        nc.tensor.matmul(dst, hsel_sb[:], sq[:],
                                         start=True, stop=True)
                    return f

                for ci in range(NQC + 1):  # 4 q chunks then k
                    w_sb = wq_sb if ci < NQC else wk_sb
                    fsl = slice(ci * P, (ci + 1) * P) if ci < NQC else slice(0, FKV)
                    ops.append(start_chunk(ci))
                    for dcc in range(NDC):
                        ops.append(mm_chunk(ci, dcc, w_sb, fsl))
                    ops.append(end_chunk(ci))

                # V^T: out[tok, feat] per 128-token chunk, accumulated over dc
                def start_v():
                    vp = psum.tile([P, TB], f32, tag="acc", name="vpack", bufs=2)
                    st["vpack"] = vp

                ops.append(start_v)

                def mm_v(tcc, dcc):
                    def f():
                        nc.tensor.matmul(
                            st["vpack"][:, tcc * P:(tcc + 1) * P],
                            st[f"xt{dcc}"][:, tcc * P:(tcc + 1) * P],
                            wv_sb[:, dcc, :],
                            start=(dcc == 0), stop=(dcc == NDC - 1))
                    return f

                for tcc in range(NTC):
                    for dcc in range(NDC):
                        ops.append(mm_v(tcc, dcc))

                def end_v():
                    vp = st["vpack"]
                    for tcc in range(NTC):
                        kc = tbn * NTC + tcc
                        nc.vector.tensor_copy(v_sb[:, kc, 0, 0:64],
                                              vp[:, tcc * P:tcc * P + 64])
                        nc.vector.tensor_copy(v_sb[:, kc, 1, 0:64],
                                              vp[:, tcc * P + 64:(tcc + 1) * P])

                ops.append(end_v)
                return st, ops

            # ---------------------------------------------------------------
            # B-rope: grouped rsqrt + RoPE for block tbn (after feed drained)
            # ---------------------------------------------------------------
            qts_cur = {}

            def emit_rope(tbn, st):
                tbs = slice(tbn * TB, (tbn + 1) * TB)
                ssq = st["ssq"]
                # grouped sqrt(ms + eps) on ACT, then reciprocal on DVE
                rrs = []
                for ci in range(NQC + 1):
                    src = (ssq[32 * ci:32 * ci + 2, :] if ci < 3
                           else st["ssk"][32 * (ci - 3):32 * (ci - 3) + 2, :])
                    sst = bpool.tile([2, TB], f32, tag="sst", name=f"sst{ci}", bufs=6)
                    nc.scalar.activation(sst[:], src,
                                         AF.Sqrt, bias=EPS, scale=1.0 / HD)
                    rrs.append(sst)
                rcs = []
                for ci in range(NQC + 1):
                    rr = bpool.tile([2, TB], f32, tag="rr", name=f"rr{ci}", bufs=6)
                    nc.vector.reciprocal_approx_fast(rr[:], rrs[ci][:])
                    rrb = bpool.tile([2, TB], bf16, tag="rrb", name=f"rrb{ci}",
                                     bufs=6)
                    nc.vector.tensor_copy(rrb[:], rr[:])
                    rcs.append(rrb)

                for ci in range(NQC + 1):
                    is_k = ci == NQC
                    rT = rkT_sb if is_k else rqT_sb
                    ct = st["ck"] if is_k else st["cq"]
                    sn_t = st["sn"]
                    qsb = st[f"qsb{ci}"]
                    bc = psum.tile([P, TB], f32, tag="sps", name=f"bc{ci}", bufs=3)
                    nc.tensor.matmul(bc[:], hexp_sb[:], rcs[ci][:],
                                     start=True, stop=True)
                    qn = bpool.tile([P, TB], bf16, tag="qn", name=f"qn{ci}", bufs=3)
                    nc.vector.tensor_tensor(qn[:], qsb[:], bc[:], ALU.mult)
                    rot = psum.tile([P, TB], f32, tag="sps", name=f"rot{ci}", bufs=3)
                    nc.tensor.matmul(rot[:], rT[:], qn[:], start=True, stop=True)
                    m1 = bpool.tile([P, TB], bf16, tag="m1", name=f"m1_{ci}", bufs=2)
                    nc.vector.tensor_tensor(m1[:], qn[:], ct[:], ALU.mult)
                    m2 = bpool.tile([P, TB], bf16, tag="m2", name=f"m2_{ci}", bufs=2)
                    nc.vector.tensor_tensor(m2[:], rot[:], sn_t[:], ALU.mult)
                    if not is_k:
                        qt = qpool.tile([P, TB], bf16, tag=f"qt{ci}", name=f"qt{ci}")
                        nc.vector.tensor_tensor(qt[:], m1[:], m2[:], ALU.add)
                        qts_cur[ci] = qt
                    else:
                        nc.vector.tensor_tensor(ktf[:, tbs], m1[:], m2[:], ALU.add)
                        nc.vector.tensor_copy(kts[0:64, tbs], ktf[64:P, tbs])
                        nc.vector.tensor_copy(kts[64:P, tbs], ktf[0:64, tbs])

            # ---------------------------------------------------------------
            # D: attention for query block tb, draining `feed` into PE gaps
            # ---------------------------------------------------------------
            def emit_D(tb, feed_ops, rope_hook=None):
                nkc = (tb + 1) * NTC
                n_iters = KVL * GROUPS * nkc
                qts = dict(qts_cur)   # rope_hook rebinds qts_cur for tb+1
                fi = 0

                def drain(n):
                    nonlocal fi
                    for _ in range(n):
                        if fi < len(feed_ops):
                            feed_ops[fi]()
                            fi += 1

                # DMAs (xt + trig) issue immediately; front-load the rest so
                # the feed completes ~60% through D
                drain(NDC + 1)
                per = ((len(feed_ops) + int(n_iters * 0.6)) // max(1, int(n_iters * 0.6))
                       if n_iters else 0)

                for g in range(KVL):
                    for pj in range(2):
                        if g == 1 and pj == 1:
                            drain(len(feed_ops))
                            if rope_hook is not None:
                                rope_hook()
                        o_pair = []
                        for hh in range(2):
                            hl = GROUPS * g + 2 * pj + hh
                            bq = 64 * (hl % 2)
                            cf = hl // 2
                            kt_tile = ktf if bq == 64 * g else kts
                            o_ps = psum.tile([P, TB], f32, tag="ops",
                                             name=f"ops{hl}", bufs=2)
                            o_pair.append(o_ps)
                            # diagonal band FIRST: batch 4 chunks of exp into
                            # esd, one affine_select, then the 4 deferred AVs.
                            # (diag-first keeps the GpSimd select latency off
                            # the head's accumulation tail)
                            esd = epool.tile([P, NTC, TB], bf16, tag="esd",
                                             name="esd", bufs=2)
                            for tdiag in range(NTC):
                                kc = tb * NTC + tdiag
                                ksl = slice(kc * P, (kc + 1) * P)
                                sps = psum.tile([P, TB], f32, tag="sps",
                                                name="sps", bufs=3)
                                nc.tensor.matmul(sps[:], kt_tile[bq:bq + 64, ksl],
                                                 qts[cf][bq:bq + 64, :],
                                                 start=True, stop=True)
                                nc.scalar.activation(esd[:, tdiag, :], sps[:],
                                                     AF.Exp, scale=float(SCALE))
                                drain(per)
                            # keep esd[p, t, f] where f >= p + 128*t
                            nc.gpsimd.affine_select(
                                out=esd[:], in_=esd[:],
                                pattern=[[-P, NTC], [1, TB]],
                                compare_op=ALU.is_ge, fill=0.0,
                                base=0, channel_multiplier=-1)
                            for tdiag in range(NTC):
                                kc = tb * NTC + tdiag
                                nc.tensor.matmul(o_ps[0:65, :],
                                                 v_sb[:, kc, g, 0:65],
                                                 esd[:, tdiag, :],
                                                 start=(tdiag == 0),
                                                 stop=(tb == 0 and tdiag == NTC - 1))
                            # off-diagonal blocks: exp straight to AV
                            for kc in range(tb * NTC):
                                ksl = slice(kc * P, (kc + 1) * P)
                                sps = psum.tile([P, TB], f32, tag="sps",
                                                name="sps", bufs=3)
                                nc.tensor.matmul(sps[:], kt_tile[bq:bq + 64, ksl],
                                                 qts[cf][bq:bq + 64, :],
                                                 start=True, stop=True)
                                es = epool.tile([P, TB], bf16, tag="es", name="es",
                                                bufs=8)
                                nc.scalar.activation(es[:], sps[:], AF.Exp,
                                                     scale=float(SCALE))
                                nc.tensor.matmul(o_ps[0:65, :],
                                                 v_sb[:, kc, g, 0:65], es[:],
                                                 start=False,
                                                 stop=(kc == tb * NTC - 1))
                                drain(per)
                        # normalize pair -> orhs[cf2]
                        cf2 = 2 * g + pj
                        dnA = npool.tile([1, TB], f32, tag="dn", name="dnA", bufs=4)
                        dnB = npool.tile([1, TB], f32, tag="dn", name="dnB", bufs=4)
                        nc.vector.tensor_copy(dnA[:], o_pair[0][64:65, :])
                        nc.vector.tensor_copy(dnB[:], o_pair[1][64:65, :])
                        rpA = npool.tile([1, TB], f32, tag="rp", name="rpA", bufs=4)
                        rpB = npool.tile([1, TB], f32, tag="rp", name="rpB", bufs=4)
                        nc.vector.reciprocal_approx_fast(rpA[:], dnA[:])
                        nc.vector.reciprocal_approx_fast(rpB[:], dnB[:])
                        rpAb = npool.tile([1, TB], bf16, tag="rpb", name="rpAb",
                                          bufs=4)
                        rpBb = npool.tile([1, TB], bf16, tag="rpb", name="rpBb",
                                          bufs=4)
                        nc.vector.tensor_copy(rpAb[:], rpA[:])
                        nc.vector.tensor_copy(rpBb[:], rpB[:])
                        bc2 = psum.tile([P, TB], f32, tag="sps", name="bc2",
                                        bufs=3)
                        nc.tensor.matmul(bc2[:], hexp_sb[0:1, :], rpAb[:],
                                         start=True, stop=False)
                        nc.tensor.matmul(bc2[:], e1_sb[:], rpBb[:],
                                         start=False, stop=True)
                        osb = npool.tile([P, TB], bf16, tag="osb", name="osb")
                        nc.vector.tensor_copy(osb[0:64, :], o_pair[0][0:64, :])
                        nc.vector.tensor_copy(osb[64:P, :], o_pair[1][0:64, :])
                        orhs = opool.tile([P, TB], bf16, tag=f"orhs{cf2}",
                                          name=f"orhs{cf2}")
                        nc.vector.tensor_tensor(orhs[:], osb[:], bc2[:], ALU.mult)
                        if cf2 == 0:
                            orhs_l = [None] * NQC
                            st_orhs[0] = orhs_l
                        st_orhs[0][cf2] = orhs
                drain(len(feed_ops))

            st_orhs = [None]

            # ---------------------------------------------------------------
            # E: output projection for block tb
            # ---------------------------------------------------------------
            def emit_E(tb):
                tbs = slice(tb * TB, (tb + 1) * TB)
                orhs_l = st_orhs[0]
                for dc2 in range(NDC):
                    ops_ = psum.tile([P, TB], f32, tag="acc", name="ops_", bufs=2)
                    for cf in range(NQC):
                        nc.tensor.matmul(ops_[:], wo_sb[:, cf, dc2 * P:(dc2 + 1) * P],
                                         orhs_l[cf][:], start=(cf == 0),
                                         stop=(cf == NQC - 1))
                    ob = outp.tile([P, TB], f32, tag="ob", name="ob")
                    nc.vector.tensor_copy(ob[:], ops_[:])
                    nc.gpsimd.dma_start(outT_d[dc2 * P:(dc2 + 1) * P, tbs], ob[:])

            # ---------------------------------------------------------------
            # main schedule
            # ---------------------------------------------------------------
            st0, feed0 = make_feed(0)
            for op in feed0:
                op()
            emit_rope(0, st0)
            for tb in range(NTB):
                if tb + 1 < NTB:
                    st_next, feed_next = make_feed(tb + 1)
                    hook = (lambda s=st_next, t=tb + 1: emit_rope(t, s))
                else:
                    st_next, feed_next, hook = None, [], None
                emit_D(tb, feed_next, rope_hook=hook)
                emit_E(tb)

    nc.compile()
    return nc


_NC_CACHE = None


def _get_nc():
    global _NC_CACHE
    if _NC_CACHE is None:
        _NC_CACHE = _build_nc()
    return _NC_CACHE


def _host_constants(q_scale, k_scale):
    pos = np.arange(T, dtype=np.float64)
    invf = 1.0 / (THETA ** (np.arange(0, HD, 2, dtype=np.float64) / HD))  # (32,)
    ang = pos[:, None] * invf[None, :]                                    # (T, 32)
    c = np.cos(ang)
    s = np.sin(ang)
    pidx = np.arange(P) % 32
    hidx = np.arange(P) % HD
    cosq = (c[:, pidx].T * q_scale[hidx][:, None]).astype(ml_dtypes.bfloat16)
    cosk = (c[:, pidx].T * k_scale[hidx][:, None]).astype(ml_dtypes.bfloat16)
    sin = s[:, pidx].T.astype(ml_dtypes.bfloat16)

    def rmat(scale):
        R = np.zeros((HD, HD), dtype=np.float64)
        for i in range(32):
            R[i, i + 32] = -scale[i + 32]
            R[i + 32, i] = scale[i]
        M = np.kron(np.eye(2), R)
        return np.ascontiguousarray(M.T.astype(ml_dtypes.bfloat16))

    hsel = np.zeros((P, 2), dtype=ml_dtypes.bfloat16)
    hsel[0:64, 0] = 1.0
    hsel[64:P, 1] = 1.0
    hexp = np.zeros((2, P), dtype=ml_dtypes.bfloat16)
    hexp[0, 0:64] = 1.0
    hexp[1, 64:P] = 1.0

    masks = np.zeros((P, NTC, TB), dtype=ml_dtypes.bfloat16)
    pp = np.arange(P)[:, None]
    ff = np.arange(TB)[None, :]
    for t in range(NTC):
        masks[:, t, :] = (ff >= pp + P * t).astype(ml_dtypes.bfloat16)

    return cosq, cosk, sin, rmat(q_scale), rmat(k_scale), hsel, hexp, masks


def _run(inputs, trace=False):
    x = np.asarray(inputs["x"], dtype=np.float32)
    Wq = np.asarray(inputs["Wq"], dtype=np.float32)
    Wk = np.asarray(inputs["Wk"], dtype=np.float32)
    Wv = np.asarray(inputs["Wv"], dtype=np.float32)
    Wo = np.asarray(inputs["Wo"], dtype=np.float32)
    q_scale = np.asarray(inputs["q_scale"], dtype=np.float64)
    k_scale = np.asarray(inputs["k_scale"], dtype=np.float64)

    cosq, cosk, sin, rqT, rkT, hsel, hexp, masks = _host_constants(q_scale, k_scale)

    bf = ml_dtypes.bfloat16
    in_maps = []
    for c in range(8):
        b = c // 4
        r = c % 4
        in_maps.append({
            "xT": np.ascontiguousarray(x[b].T).astype(bf),
            "wq": np.ascontiguousarray(Wq[:, r * FQ:(r + 1) * FQ]).astype(bf),
            "wk": np.ascontiguousarray(Wk[:, r * FKV:(r + 1) * FKV]).astype(bf),
            "wv": np.ascontiguousarray(Wv[:, r * FKV:(r + 1) * FKV]).astype(bf),
            "wo": np.ascontiguousarray(Wo[r * FQ:(r + 1) * FQ, :]).astype(bf),
            "cosq": cosq, "cosk": cosk, "sin": sin,
            "rqT": rqT, "rkT": rkT, "hsel": hsel,
            "hexp": hexp, "e1": np.ascontiguousarray(hexp[1:2, :]),
            "masks": masks,
        })

    nc = _get_nc()
    res = run_bass_kernel_spmd(nc, in_maps, core_ids=list(range(8)), trace=trace)
    out = np.empty((B, T, D), dtype=np.float32)
    for b in range(B):
        acc = res.results[4 * b]["outT"].astype(np.float32).copy()
        for r in range(1, 4):
            acc += res.results[4 * b + r]["outT"]
        out[b] = acc.T
    return out, res


def kernel(**inputs):
    out, _ = _run(inputs, trace=False)
    return out


# revision 44
# speedup vs baseline: 1.7702x; 1.0951x over previous
"""GQA (32 q heads / 8 kv heads, head_dim 64, causal, QK-RMSNorm + RoPE) on 8 TRN2 cores.

Sharding: data-parallel over batch (2) x tensor-parallel over heads (4):
each core handles one batch element, 8 query heads, 2 kv heads, and produces
a partial output (its heads' slice of the Wo contraction); the host sums the
4 partials per batch element.

v2: all matmuls in bf16 (PSUM accumulation stays f32), software-pipelined so
the PE never idles (projection matmuls for block tb+1 are interleaved into
the attention inner loop of block tb), RMSNorm via Sqrt+DVE-reciprocal
(no Ln/Exp table thrash), causal masks applied on GpSimd, V^T produced
directly by the projection (no PE transposes).
"""

import numpy as np
import ml_dtypes

import concourse.bass as bass
import concourse.mybir as mybir
import concourse.tile as tile
from concourse import bacc
from concourse.bass_utils import run_bass_kernel_spmd

# Problem config (hardcoded per contract)
B, T, D = 2, 2048, 2048
H, KV, HD = 32, 8, 64
GROUPS = H // KV
THETA = 10000.0
SCALE = 1.0 / np.sqrt(HD)
EPS = 1e-6

# Per-core sharding
HQL = H // 4          # 8 local q heads
KVL = KV // 4         # 2 local kv heads
FQ = HQL * HD         # 512
FKV = KVL * HD        # 128

# Tiling
P = 128
TB = 512              # token block
NTB = T // TB         # 4
NDC = D // P          # 16 contraction chunks
NKC = T // P          # 16 key chunks
NQC = FQ // P         # 4 q-proj chunks (2 heads each)
NTC = TB // P         # 4 token chunks per block

f32 = mybir.dt.float32
bf16 = mybir.dt.bfloat16
AF = mybir.ActivationFunctionType
ALU = mybir.AluOpType


def _build_nc():
    nc = bacc.Bacc("TRN2", target_bir_lowering=False, debug=False, num_devices=8)

    eps_t = nc.alloc_sbuf_tensor("const-f32-eps", [128, 1], f32)
    nc.gpsimd.memset(eps_t.ap(), EPS)
    nc.const_aps.aps[(f32, EPS)] = eps_t.ap()
    zero_t = nc.alloc_sbuf_tensor("const-f32-zero", [128, 1], f32)
    nc.gpsimd.memset(zero_t.ap(), 0.0)
    nc.const_aps.aps[(f32, 0.0)] = zero_t.ap()
    nc.all_engine_barrier()

    xT_d = nc.dram_tensor("xT", [D, T], bf16, kind="ExternalInput")
    wq_d = nc.dram_tensor("wq", [D, FQ], bf16, kind="ExternalInput")
    wk_d = nc.dram_tensor("wk", [D, FKV], bf16, kind="ExternalInput")
    wv_d = nc.dram_tensor("wv", [D, FKV], bf16, kind="ExternalInput")
    wo_d = nc.dram_tensor("wo", [FQ, D], bf16, kind="ExternalInput")
    cosq_d = nc.dram_tensor("cosq", [P, T], bf16, kind="ExternalInput")
    cosk_d = nc.dram_tensor("cosk", [P, T], bf16, kind="ExternalInput")
    sin_d = nc.dram_tensor("sin", [P, T], bf16, kind="ExternalInput")
    rqT_d = nc.dram_tensor("rqT", [P, P], bf16, kind="ExternalInput")
    rkT_d = nc.dram_tensor("rkT", [P, P], bf16, kind="ExternalInput")
    hsel_d = nc.dram_tensor("hsel", [P, 2], bf16, kind="ExternalInput")
    hexp_d = nc.dram_tensor("hexp", [2, P], bf16, kind="ExternalInput")
    e1_d = nc.dram_tensor("e1", [1, P], bf16, kind="ExternalInput")
    masks_d = nc.dram_tensor("masks", [P, NTC, TB], bf16, kind="ExternalInput")
    outT_d = nc.dram_tensor("outT", [D, T], f32, kind="ExternalOutput")

    with tile.TileContext(nc) as tc:
        with (
            tc.tile_pool(name="wpool", bufs=1) as wpool,
            tc.tile_pool(name="cpool", bufs=1) as cpool,
            tc.tile_pool(name="kvpool", bufs=1) as kvpool,
            tc.tile_pool(name="xpool", bufs=18) as xpool,
            tc.tile_pool(name="trig", bufs=2) as trig,
            tc.tile_pool(name="bpool", bufs=6) as bpool,
            tc.tile_pool(name="qpool", bufs=2) as qpool,
            tc.tile_pool(name="epool", bufs=6) as epool,
            tc.tile_pool(name="npool", bufs=2) as npool,
            tc.tile_pool(name="opool", bufs=2) as opool,
            tc.tile_pool(name="outp", bufs=3) as outp,
            tc.tile_pool(name="psum", bufs=1, space="PSUM") as psum,
        ):
            # ---- persistent weights / constants ----
            wq_sb = wpool.tile([P, NDC, FQ], bf16)
            wk_sb = wpool.tile([P, NDC, FKV], bf16)
            wv_sb = wpool.tile([P, NDC, FKV], bf16)
            wo_sb = wpool.tile([P, NQC, D], bf16)
            nc.sync.dma_start(wq_sb[:], wq_d.rearrange("(ko p) f -> p ko f", p=P))
            nc.sync.dma_start(wk_sb[:], wk_d.rearrange("(ko p) f -> p ko f", p=P))
            nc.sync.dma_start(wv_sb[:], wv_d.rearrange("(ko p) f -> p ko f", p=P))
            nc.sync.dma_start(wo_sb[:], wo_d.rearrange("(ko p) f -> p ko f", p=P))

            rqT_sb = cpool.tile([P, P], bf16)
            rkT_sb = cpool.tile([P, P], bf16)
            hsel_sb = cpool.tile([P, 2], bf16)
            hexp_sb = cpool.tile([2, P], bf16)
            e1_sb = cpool.tile([1, P], bf16)
            masks_sb = cpool.tile([P, NTC, TB], bf16)
            nc.sync.dma_start(rqT_sb[:], rqT_d[:])
            nc.sync.dma_start(rkT_sb[:], rkT_d[:])
            nc.sync.dma_start(hsel_sb[:], hsel_d[:])
            nc.sync.dma_start(hexp_sb[:], hexp_d[:])
            nc.sync.dma_start(e1_sb[:], e1_d[:])
            nc.sync.dma_start(masks_sb[:], masks_d[:])

            # K^T per-kv-head at both partition placements, V (+ones col)
            ktf = kvpool.tile([P, T], bf16)          # rows 0:64 kv0, 64:128 kv1
            kts = kvpool.tile([P, T], bf16)          # swapped halves
            v_sb = kvpool.tile([P, NKC, KVL, 66], bf16)  # [tok, kc, g, hd+ones+pad]
            ones_bc = nc.const_aps.tensor(1.0, (P, NKC, KVL, 66), bf16)
            nc.vector.tensor_copy(v_sb[:], ones_bc)

            # ---------------------------------------------------------------
            # Feed: projection + square/copy work for token block tbn,
            # returned as a list of closures to be drained into D(tbn-1).
            # ---------------------------------------------------------------
            def make_feed(tbn):
                tbs = slice(tbn * TB, (tbn + 1) * TB)
                st = {}
                ops = []

                def load_xt():
                    xt = xpool.tile([P, NDC, TB], bf16, tag="xt", name="xt",
                                    bufs=2)
                    nc.sync.dma_start(
                        xt[:],
                        xT_d.rearrange("(ko p) t -> p ko t", p=P)[:, :, tbs])
                    st["xt"] = xt

                ops.append(load_xt)

                def load_trig():
                    cq_t = trig.tile([P, TB], bf16, tag="cq", name="cq_t")
                    ck_t = trig.tile([P, TB], bf16, tag="ck", name="ck_t")
                    sn_t = trig.tile([P, TB], bf16, tag="sn", name="sn_t")
                    nc.sync.dma_start(cq_t[:], cosq_d[:, tbs])
                    nc.sync.dma_start(ck_t[:], cosk_d[:, tbs])
                    nc.sync.dma_start(sn_t[:], sin_d[:, tbs])
                    st["cq"], st["ck"], st["sn"] = cq_t, ck_t, sn_t

                ops.append(load_trig)

                # q0..q3 and k projection chunks (chunk-major over dc)
                def start_chunk(ci):
                    def f():
                        acc = psum.tile([P, TB], f32, tag="acc", name=f"acc{ci}",
                                        bufs=2)
                        st["acc"] = acc
                    return f

                def mm_chunk(ci, dcc, w_sb, fsl):
                    def f():
                        nc.tensor.matmul(st["acc"][:], w_sb[:, dcc, fsl],
                                         st["xt"][:, dcc, :],
                                         start=(dcc == 0), stop=(dcc == NDC - 1))
                    return f

                def end_chunk(ci):
                    # bf16 copy for RoPE, square on DVE, packed sumsq matmul
                    def f():
                        acc = st["acc"]
                        qsb = bpool.tile([P, TB], bf16, tag="qsb", name=f"qsb{ci}")
                        nc.vector.tensor_copy(qsb[:], acc[:])
                        st[f"qsb{ci}"] = qsb
                        sq = bpool.tile([P, TB], bf16, tag="sq", name=f"sq{ci}")
                        nc.vector.tensor_tensor(sq[:], qsb[:], qsb[:], ALU.mult)
                        if ci == 0:
                            ssq = psum.tile([P, TB], f32, tag="ss", name="ssq")
                            st["ssq"] = ssq
                        if ci == 3:
                            ssk = psum.tile([P, TB], f32, tag="sps", name="ssk",
                                            bufs=3)
                            st["ssk"] = ssk
                        # base partition of a PSUM out AP must be 0/32/64:
                        # pack q0-q2 in ssq, q3+k in ssk (sps ring, short-lived)
                        dst = (st["ssq"][32 * ci:32 * ci + 2, :] if ci < 3
                               else st["ssk"][32 * (ci - 3):32 * (ci - 3) + 2, :])
                        nc.tensor.matmul(dst, hsel_sb[:], sq[:],
                                         start=True, stop=True)
                    return f

                for ci in range(NQC + 1):  # 4 q chunks then k
                    w_sb = wq_sb if ci < NQC else wk_sb
                    fsl = slice(ci * P, (ci + 1) * P) if ci < NQC else slice(0, FKV)
                    ops.append(start_chunk(ci))
                    for dcc in range(NDC):
                        ops.append(mm_chunk(ci, dcc, w_sb, fsl))
                    ops.append(end_chunk(ci))

                # V^T: out[tok, feat] per 128-token chunk, accumulated over dc
                def start_v():
                    vp = psum.tile([P, TB], f32, tag="acc", name="vpack", bufs=2)
                    st["vpack"] = vp

                ops.append(start_v)

                def mm_v(tcc, dcc):
                    def f():
                        nc.tensor.matmul(
                            st["vpack"][:, tcc * P:(tcc + 1) * P],
                            st["xt"][:, dcc, tcc * P:(tcc + 1) * P],
                            wv_sb[:, dcc, :],
                            start=(dcc == 0), stop=(dcc == NDC - 1))
                    return f

                for tcc in range(NTC):
                    for dcc in range(NDC):
                        ops.append(mm_v(tcc, dcc))

                def end_v():
                    vp = st["vpack"]
                    for tcc in range(NTC):
                        kc = tbn * NTC + tcc
                        nc.vector.tensor_copy(v_sb[:, kc, 0, 0:64],
                                              vp[:, tcc * P:tcc * P + 64])
                        nc.vector.tensor_copy(v_sb[:, kc, 1, 0:64],
                                              vp[:, tcc * P + 64:(tcc + 1) * P])

                ops.append(end_v)
                return st, ops

            # ---------------------------------------------------------------
            # B-rope: grouped rsqrt + RoPE for block tbn (after feed drained)
            # ---------------------------------------------------------------
            qts_cur = {}

            def emit_rope(tbn, st):
                tbs = slice(tbn * TB, (tbn + 1) * TB)
                ssq = st["ssq"]
                # grouped sqrt(ms + eps) on ACT, then reciprocal on DVE
                rrs = []
                for ci in range(NQC + 1):
                    src = (ssq[32 * ci:32 * ci + 2, :] if ci < 3
                           else st["ssk"][32 * (ci - 3):32 * (ci - 3) + 2, :])
                    sst = bpool.tile([2, TB], f32, tag="sst", name=f"sst{ci}", bufs=6)
                    nc.scalar.activation(sst[:], src,
                                         AF.Sqrt, bias=EPS, scale=1.0 / HD)
                    rrs.append(sst)
                rcs = []
                for ci in range(NQC + 1):
                    rr = bpool.tile([2, TB], f32, tag="rr", name=f"rr{ci}", bufs=6)
                    nc.vector.reciprocal_approx_fast(rr[:], rrs[ci][:])
                    rrb = bpool.tile([2, TB], bf16, tag="rrb", name=f"rrb{ci}",
                                     bufs=6)
                    nc.vector.tensor_copy(rrb[:], rr[:])
                    rcs.append(rrb)

                for ci in range(NQC + 1):
                    is_k = ci == NQC
                    rT = rkT_sb if is_k else rqT_sb
                    ct = st["ck"] if is_k else st["cq"]
                    sn_t = st["sn"]
                    qsb = st[f"qsb{ci}"]
                    bc = psum.tile([P, TB], f32, tag="sps", name=f"bc{ci}", bufs=3)
                    nc.tensor.matmul(bc[:], hexp_sb[:], rcs[ci][:],
                                     start=True, stop=True)
                    qn = bpool.tile([P, TB], bf16, tag="qn", name=f"qn{ci}", bufs=3)
                    nc.vector.tensor_tensor(qn[:], qsb[:], bc[:], ALU.mult)
                    rot = psum.tile([P, TB], f32, tag="sps", name=f"rot{ci}", bufs=3)
                    nc.tensor.matmul(rot[:], rT[:], qn[:], start=True, stop=True)
                    m1 = bpool.tile([P, TB], bf16, tag="m1", name=f"m1_{ci}", bufs=2)
                    nc.vector.tensor_tensor(m1[:], qn[:], ct[:], ALU.mult)
                    m2 = bpool.tile([P, TB], bf16, tag="m2", name=f"m2_{ci}", bufs=2)
                    nc.vector.tensor_tensor(m2[:], rot[:], sn_t[:], ALU.mult)
                    if not is_k:
                        qt = qpool.tile([P, TB], bf16, tag=f"qt{ci}", name=f"qt{ci}")
                        nc.vector.tensor_tensor(qt[:], m1[:], m2[:], ALU.add)
                        qts_cur[ci] = qt
                    else:
                        nc.vector.tensor_tensor(ktf[:, tbs], m1[:], m2[:], ALU.add)
                        nc.vector.tensor_copy(kts[0:64, tbs], ktf[64:P, tbs])
                        nc.vector.tensor_copy(kts[64:P, tbs], ktf[0:64, tbs])

            # ---------------------------------------------------------------
            # D: attention for query block tb, draining `feed` into PE gaps
            # ---------------------------------------------------------------
            def emit_D(tb, feed_ops, e_ops, rope_hook=None):
                nkc = (tb + 1) * NTC
                n_iters = KVL * GROUPS * nkc
                qts = dict(qts_cur)   # rope_hook rebinds qts_cur for tb+1
                fi = 0
                ei = 0
                it = 0

                def drain(n):
                    nonlocal fi
                    for _ in range(n):
                        if fi < len(feed_ops):
                            feed_ops[fi]()
                            fi += 1

                e_every = max(1, n_iters // (len(e_ops) + 1)) if e_ops else 0

                def tick():
                    nonlocal ei, it
                    it += 1
                    if e_ops and ei < len(e_ops) and it % e_every == 0:
                        e_ops[ei]()
                        ei += 1

                # DMAs (xt + trig) issue immediately; front-load the rest so
                # the feed completes ~60% through D
                drain(2)
                per = ((len(feed_ops) + int(n_iters * 0.6)) // max(1, int(n_iters * 0.6))
                       if n_iters else 0)

                for g in range(KVL):
                    for pj in range(2):
                        if g == 1 and pj == 1:
                            drain(len(feed_ops))
                            if rope_hook is not None:
                                rope_hook()
                        o_pair = []
                        for hh in range(2):
                            hl = GROUPS * g + 2 * pj + hh
                            bq = 64 * (hl % 2)
                            cf = hl // 2
                            kt_tile = ktf if bq == 64 * g else kts
                            o_ps = psum.tile([P, TB], f32, tag="ops",
                                             name=f"ops{hl}", bufs=2)
                            o_pair.append(o_ps)
                            # diagonal band FIRST: batch 4 chunks of exp into
                            # esd, one affine_select, then the 4 deferred AVs.
                            # (diag-first keeps the GpSimd select latency off
                            # the head's accumulation tail)
                            esd = epool.tile([P, NTC, TB], bf16, tag="esd",
                                             name="esd", bufs=2)
                            for tdiag in range(NTC):
                                kc = tb * NTC + tdiag
                                ksl = slice(kc * P, (kc + 1) * P)
                                sps = psum.tile([P, TB], f32, tag="sps",
                                                name="sps", bufs=3)
                                nc.tensor.matmul(sps[:], kt_tile[bq:bq + 64, ksl],
                                                 qts[cf][bq:bq + 64, :],
                                                 start=True, stop=True)
                                nc.scalar.activation(esd[:, tdiag, :], sps[:],
                                                     AF.Exp, scale=float(SCALE))
                                drain(per)
                                tick()
                            # keep esd[p, t, f] where f >= p + 128*t
                            nc.gpsimd.affine_select(
                                out=esd[:], in_=esd[:],
                                pattern=[[-P, NTC], [1, TB]],
                                compare_op=ALU.is_ge, fill=0.0,
                                base=0, channel_multiplier=-1)
                            for tdiag in range(NTC):
                                kc = tb * NTC + tdiag
                                nc.tensor.matmul(o_ps[0:65, :],
                                                 v_sb[:, kc, g, 0:65],
                                                 esd[:, tdiag, :],
                                                 start=(tdiag == 0),
                                                 stop=(tb == 0 and tdiag == NTC - 1))
                            # off-diagonal blocks: exp straight to AV
                            for kc in range(tb * NTC):
                                ksl = slice(kc * P, (kc + 1) * P)
                                sps = psum.tile([P, TB], f32, tag="sps",
                                                name="sps", bufs=3)
                                nc.tensor.matmul(sps[:], kt_tile[bq:bq + 64, ksl],
                                                 qts[cf][bq:bq + 64, :],
                                                 start=True, stop=True)
                                es = epool.tile([P, TB], bf16, tag="es", name="es",
                                                bufs=8)
                                nc.scalar.activation(es[:], sps[:], AF.Exp,
                                                     scale=float(SCALE))
                                nc.tensor.matmul(o_ps[0:65, :],
                                                 v_sb[:, kc, g, 0:65], es[:],
                                                 start=False,
                                                 stop=(kc == tb * NTC - 1))
                                drain(per)
                                tick()
                        # normalize pair -> orhs[cf2]
                        cf2 = 2 * g + pj
                        dnA = npool.tile([1, TB], f32, tag="dn", name="dnA", bufs=4)
                        dnB = npool.tile([1, TB], f32, tag="dn", name="dnB", bufs=4)
                        nc.vector.tensor_copy(dnA[:], o_pair[0][64:65, :])
                        nc.vector.tensor_copy(dnB[:], o_pair[1][64:65, :])
                        rpA = npool.tile([1, TB], f32, tag="rp", name="rpA", bufs=4)
                        rpB = npool.tile([1, TB], f32, tag="rp", name="rpB", bufs=4)
                        nc.vector.reciprocal_approx_fast(rpA[:], dnA[:])
                        nc.vector.reciprocal_approx_fast(rpB[:], dnB[:])
                        rpAb = npool.tile([1, TB], bf16, tag="rpb", name="rpAb",
                                          bufs=4)
                        rpBb = npool.tile([1, TB], bf16, tag="rpb", name="rpBb",
                                          bufs=4)
                        nc.vector.tensor_copy(rpAb[:], rpA[:])
                        nc.vector.tensor_copy(rpBb[:], rpB[:])
                        bc2 = psum.tile([P, TB], f32, tag="sps", name="bc2",
                                        bufs=3)
                        nc.tensor.matmul(bc2[:], hexp_sb[0:1, :], rpAb[:],
                                         start=True, stop=False)
                        nc.tensor.matmul(bc2[:], e1_sb[:], rpBb[:],
                                         start=False, stop=True)
                        osb = npool.tile([P, TB], bf16, tag="osb", name="osb")
                        nc.vector.tensor_copy(osb[0:64, :], o_pair[0][0:64, :])
                        nc.vector.tensor_copy(osb[64:P, :], o_pair[1][0:64, :])
                        orhs = opool.tile([P, TB], bf16, tag=f"orhs{cf2}",
                                          name=f"orhs{cf2}")
                        nc.vector.tensor_tensor(orhs[:], osb[:], bc2[:], ALU.mult)
                        if cf2 == 0:
                            orhs_l = [None] * NQC
                            st_orhs[0] = orhs_l
                        st_orhs[0][cf2] = orhs
                drain(len(feed_ops))
                while e_ops and ei < len(e_ops):
                    e_ops[ei]()
                    ei += 1

            st_orhs = [None]

            # ---------------------------------------------------------------
            # E: output projection for block tb
            # ---------------------------------------------------------------
            def make_E(tb):
                tbs = slice(tb * TB, (tb + 1) * TB)
                orhs_l = st_orhs[0]

                def blk(dc2):
                    def f():
                        ops_ = psum.tile([P, TB], f32, tag="acc", name="ops_",
                                         bufs=2)
                        for cf in range(NQC):
                            nc.tensor.matmul(
                                ops_[:], wo_sb[:, cf, dc2 * P:(dc2 + 1) * P],
                                orhs_l[cf][:], start=(cf == 0),
                                stop=(cf == NQC - 1))
                        ob = outp.tile([P, TB], f32, tag="ob", name="ob")
                        nc.vector.tensor_copy(ob[:], ops_[:])
                        nc.gpsimd.dma_start(outT_d[dc2 * P:(dc2 + 1) * P, tbs],
                                            ob[:])
                    return f

                return [blk(dc2) for dc2 in range(NDC)]

            # ---------------------------------------------------------------
            # main schedule: D(tb) drains the A/B feed for tb+1 plus the
            # output projection of tb-1 into the PE's exp-wait gaps.
            # ---------------------------------------------------------------
            st0, feed0 = make_feed(0)
            for op in feed0:
                op()
            emit_rope(0, st0)
            e_prev = []
            for tb in range(NTB):
                if tb + 1 < NTB:
                    st_next, feed_next = make_feed(tb + 1)
                    hook = (lambda s=st_next, t=tb + 1: emit_rope(t, s))
                else:
                    st_next, feed_next, hook = None, [], None
                emit_D(tb, feed_next, e_prev, rope_hook=hook)
                e_prev = make_E(tb)
            for op in e_prev:
                op()

    nc.compile()
    return nc


_NC_CACHE = None


def _get_nc():
    global _NC_CACHE
    if _NC_CACHE is None:
        _NC_CACHE = _build_nc()
    return _NC_CACHE


def _host_constants(q_scale, k_scale):
    pos = np.arange(T, dtype=np.float64)
    invf = 1.0 / (THETA ** (np.arange(0, HD, 2, dtype=np.float64) / HD))  # (32,)
    ang = pos[:, None] * invf[None, :]                                    # (T, 32)
    c = np.cos(ang)
    s = np.sin(ang)
    pidx = np.arange(P) % 32
    hidx = np.arange(P) % HD
    cosq = (c[:, pidx].T * q_scale[hidx][:, None]).astype(ml_dtypes.bfloat16)
    cosk = (c[:, pidx].T * k_scale[hidx][:, None]).astype(ml_dtypes.bfloat16)
    sin = s[:, pidx].T.astype(ml_dtypes.bfloat16)

    def rmat(scale):
        R = np.zeros((HD, HD), dtype=np.float64)
        for i in range(32):
            R[i, i + 32] = -scale[i + 32]
            R[i + 32, i] = scale[i]
        M = np.kron(np.eye(2), R)
        return np.ascontiguousarray(M.T.astype(ml_dtypes.bfloat16))

    hsel = np.zeros((P, 2), dtype=ml_dtypes.bfloat16)
    hsel[0:64, 0] = 1.0
    hsel[64:P, 1] = 1.0
    hexp = np.zeros((2, P), dtype=ml_dtypes.bfloat16)
    hexp[0, 0:64] = 1.0
    hexp[1, 64:P] = 1.0

    masks = np.zeros((P, NTC, TB), dtype=ml_dtypes.bfloat16)
    pp = np.arange(P)[:, None]
    ff = np.arange(TB)[None, :]
    for t in range(NTC):
        masks[:, t, :] = (ff >= pp + P * t).astype(ml_dtypes.bfloat16)

    return cosq, cosk, sin, rmat(q_scale), rmat(k_scale), hsel, hexp, masks


def _run(inputs, trace=False):
    x = np.asarray(inputs["x"], dtype=np.float32)
    Wq = np.asarray(inputs["Wq"], dtype=np.float32)
    Wk = np.asarray(inputs["Wk"], dtype=np.float32)
    Wv = np.asarray(inputs["Wv"], dtype=np.float32)
    Wo = np.asarray(inputs["Wo"], dtype=np.float32)
    q_scale = np.asarray(inputs["q_scale"], dtype=np.float64)
    k_scale = np.asarray(inputs["k_scale"], dtype=np.float64)

    cosq, cosk, sin, rqT, rkT, hsel, hexp, masks = _host_constants(q_scale, k_scale)

    bf = ml_dtypes.bfloat16
    in_maps = []
    for c in range(8):
        b = c // 4
        r = c % 4
        in_maps.append({
            "xT": np.ascontiguousarray(x[b].T).astype(bf),
            "wq": np.ascontiguousarray(Wq[:, r * FQ:(r + 1) * FQ]).astype(bf),
            "wk": np.ascontiguousarray(Wk[:, r * FKV:(r + 1) * FKV]).astype(bf),
            "wv": np.ascontiguousarray(Wv[:, r * FKV:(r + 1) * FKV]).astype(bf),
            "wo": np.ascontiguousarray(Wo[r * FQ:(r + 1) * FQ, :]).astype(bf),
            "cosq": cosq, "cosk": cosk, "sin": sin,
            "rqT": rqT, "rkT": rkT, "hsel": hsel,
            "hexp": hexp, "e1": np.ascontiguousarray(hexp[1:2, :]),
            "masks": masks,
        })

    nc = _get_nc()
    res = run_bass_kernel_spmd(nc, in_maps, core_ids=list(range(8)), trace=trace)
    out = np.empty((B, T, D), dtype=np.float32)
    for b in range(B):
        acc = res.results[4 * b]["outT"].astype(np.float32).copy()
        for r in range(1, 4):
            acc += res.results[4 * b + r]["outT"]
        out[b] = acc.T
    return out, res


def kernel(**inputs):
    out, _ = _run(inputs, trace=False)
    return out


# revision 47
# speedup vs baseline: 1.8209x; 1.0286x over previous
"""GQA (32 q heads / 8 kv heads, head_dim 64, causal, QK-RMSNorm + RoPE) on 8 TRN2 cores.

Sharding: data-parallel over batch (2) x tensor-parallel over heads (4):
each core handles one batch element, 8 query heads, 2 kv heads, and produces
a partial output (its heads' slice of the Wo contraction); the host sums the
4 partials per batch element.

v2: all matmuls in bf16 (PSUM accumulation stays f32), software-pipelined so
the PE never idles (projection matmuls for block tb+1 are interleaved into
the attention inner loop of block tb), RMSNorm via Sqrt+DVE-reciprocal
(no Ln/Exp table thrash), causal masks applied on GpSimd, V^T produced
directly by the projection (no PE transposes).
"""

import numpy as np
import ml_dtypes

import concourse.bass as bass
import concourse.mybir as mybir
import concourse.tile as tile
from concourse import bacc
from concourse.bass_utils import run_bass_kernel_spmd

# Problem config (hardcoded per contract)
B, T, D = 2, 2048, 2048
H, KV, HD = 32, 8, 64
GROUPS = H // KV
THETA = 10000.0
SCALE = 1.0 / np.sqrt(HD)
EPS = 1e-6

# Per-core sharding
HQL = H // 4          # 8 local q heads
KVL = KV // 4         # 2 local kv heads
FQ = HQL * HD         # 512
FKV = KVL * HD        # 128

# Tiling
P = 128
TB = 512              # token block
NTB = T // TB         # 4
NDC = D // P          # 16 contraction chunks
NKC = T // P          # 16 key chunks
NQC = FQ // P         # 4 q-proj chunks (2 heads each)
NTC = TB // P         # 4 token chunks per block

f32 = mybir.dt.float32
bf16 = mybir.dt.bfloat16
AF = mybir.ActivationFunctionType
ALU = mybir.AluOpType


def _build_nc():
    nc = bacc.Bacc("TRN2", target_bir_lowering=False, debug=False, num_devices=8)

    eps_t = nc.alloc_sbuf_tensor("const-f32-eps", [128, 1], f32)
    nc.gpsimd.memset(eps_t.ap(), EPS)
    nc.const_aps.aps[(f32, EPS)] = eps_t.ap()
    zero_t = nc.alloc_sbuf_tensor("const-f32-zero", [128, 1], f32)
    nc.gpsimd.memset(zero_t.ap(), 0.0)
    nc.const_aps.aps[(f32, 0.0)] = zero_t.ap()
    nc.all_engine_barrier()

    xT_d = nc.dram_tensor("xT", [D, T], bf16, kind="ExternalInput")
    wq_d = nc.dram_tensor("wq", [D, FQ], bf16, kind="ExternalInput")
    wk_d = nc.dram_tensor("wk", [D, FKV], bf16, kind="ExternalInput")
    wv_d = nc.dram_tensor("wv", [D, FKV], bf16, kind="ExternalInput")
    wo_d = nc.dram_tensor("wo", [FQ, D], bf16, kind="ExternalInput")
    cosq_d = nc.dram_tensor("cosq", [P, T], bf16, kind="ExternalInput")
    cosk_d = nc.dram_tensor("cosk", [P, T], bf16, kind="ExternalInput")
    sin_d = nc.dram_tensor("sin", [P, T], bf16, kind="ExternalInput")
    rqT_d = nc.dram_tensor("rqT", [P, P], bf16, kind="ExternalInput")
    rkT_d = nc.dram_tensor("rkT", [P, P], bf16, kind="ExternalInput")
    hsel_d = nc.dram_tensor("hsel", [P, 2], bf16, kind="ExternalInput")
    hexp_d = nc.dram_tensor("hexp", [2, P], bf16, kind="ExternalInput")
    e1_d = nc.dram_tensor("e1", [1, P], bf16, kind="ExternalInput")
    outT_d = nc.dram_tensor("outT", [D, T], f32, kind="ExternalOutput")

    with tile.TileContext(nc) as tc:
        with (
            tc.tile_pool(name="wpool", bufs=1) as wpool,
            tc.tile_pool(name="cpool", bufs=1) as cpool,
            tc.tile_pool(name="kvpool", bufs=1) as kvpool,
            tc.tile_pool(name="xpool", bufs=2) as xpool,
            tc.tile_pool(name="trig", bufs=2) as trig,
            tc.tile_pool(name="bpool", bufs=6) as bpool,
            tc.tile_pool(name="qpool", bufs=2) as qpool,
            tc.tile_pool(name="epool", bufs=6) as epool,
            tc.tile_pool(name="npool", bufs=2) as npool,
            tc.tile_pool(name="opool", bufs=2) as opool,
            tc.tile_pool(name="outp", bufs=2) as outp,
            tc.tile_pool(name="psum", bufs=1, space="PSUM") as psum,
        ):
            # ---- persistent weights / constants ----
            wq_sb = wpool.tile([P, NDC, FQ], bf16)
            wk_sb = wpool.tile([P, NDC, FKV], bf16)
            wv_sb = wpool.tile([P, NDC, FKV], bf16)
            wo_sb = wpool.tile([P, NQC, D], bf16)
            nc.sync.dma_start(wq_sb[:], wq_d.rearrange("(ko p) f -> p ko f", p=P))
            nc.sync.dma_start(wk_sb[:], wk_d.rearrange("(ko p) f -> p ko f", p=P))
            nc.sync.dma_start(wv_sb[:], wv_d.rearrange("(ko p) f -> p ko f", p=P))
            nc.sync.dma_start(wo_sb[:], wo_d.rearrange("(ko p) f -> p ko f", p=P))

            rqT_sb = cpool.tile([P, P], bf16)
            rkT_sb = cpool.tile([P, P], bf16)
            hsel_sb = cpool.tile([P, 2], bf16)
            hexp_sb = cpool.tile([2, P], bf16)
            e1_sb = cpool.tile([1, P], bf16)
            nc.sync.dma_start(rqT_sb[:], rqT_d[:])
            nc.sync.dma_start(rkT_sb[:], rkT_d[:])
            nc.sync.dma_start(hsel_sb[:], hsel_d[:])
            nc.sync.dma_start(hexp_sb[:], hexp_d[:])
            nc.sync.dma_start(e1_sb[:], e1_d[:])

            # K^T per-kv-head at both partition placements, V (+ones col)
            ktf = kvpool.tile([P, T], bf16)          # rows 0:64 kv0, 64:128 kv1
            kts = kvpool.tile([P, T], bf16)          # swapped halves
            v_sb = kvpool.tile([P, NKC, KVL, 66], bf16)  # [tok, kc, g, hd+ones+pad]
            ones_bc = nc.const_aps.tensor(1.0, (P, NKC, KVL, 66), bf16)
            nc.vector.tensor_copy(v_sb[:], ones_bc)

            # ---------------------------------------------------------------
            # Feed: projection + square/copy work for token block tbn,
            # returned as a list of closures to be drained into D(tbn-1).
            # ---------------------------------------------------------------
            def make_feed(tbn):
                tbs = slice(tbn * TB, (tbn + 1) * TB)
                st = {}
                ops = []

                def load_xt():
                    xt = xpool.tile([P, NDC, TB], bf16, tag="xt", name="xt",
                                    bufs=2)
                    xr = xT_d.rearrange("(ko p) t -> p ko t", p=P)
                    for q4 in range(4):
                        nc.sync.dma_start(xt[:, 4 * q4:4 * (q4 + 1), :],
                                          xr[:, 4 * q4:4 * (q4 + 1), tbs])
                    st["xt"] = xt

                ops.append(load_xt)

                def load_trig():
                    cq_t = trig.tile([P, TB], bf16, tag="cq", name="cq_t")
                    ck_t = trig.tile([P, TB], bf16, tag="ck", name="ck_t")
                    sn_t = trig.tile([P, TB], bf16, tag="sn", name="sn_t")
                    nc.sync.dma_start(cq_t[:], cosq_d[:, tbs])
                    nc.sync.dma_start(ck_t[:], cosk_d[:, tbs])
                    nc.sync.dma_start(sn_t[:], sin_d[:, tbs])
                    st["cq"], st["ck"], st["sn"] = cq_t, ck_t, sn_t

                ops.append(load_trig)

                # q0..q3 and k projection chunks (chunk-major over dc)
                def start_chunk(ci):
                    def f():
                        acc = psum.tile([P, TB], f32, tag="acc", name=f"acc{ci}",
                                        bufs=2)
                        st["acc"] = acc
                    return f

                def mm_chunk(ci, dcc, w_sb, fsl):
                    def f():
                        nc.tensor.matmul(st["acc"][:], w_sb[:, dcc, fsl],
                                         st["xt"][:, dcc, :],
                                         start=(dcc == 0), stop=(dcc == NDC - 1))
                    return f

                def end_chunk(ci):
                    # bf16 copy for RoPE, square on DVE, packed sumsq matmul
                    def f():
                        acc = st["acc"]
                        qsb = bpool.tile([P, TB], bf16, tag="qsb", name=f"qsb{ci}")
                        nc.vector.tensor_copy(qsb[:], acc[:])
                        st[f"qsb{ci}"] = qsb
                        sq = bpool.tile([P, TB], bf16, tag="sq", name=f"sq{ci}")
                        nc.vector.tensor_tensor(sq[:], qsb[:], qsb[:], ALU.mult)
                        ssp = psum.tile([P, TB], f32, tag="sps", name="ssp",
                                        bufs=4)
                        nc.tensor.matmul(ssp[0:2, :], hsel_sb[:], sq[:],
                                         start=True, stop=True)
                        ssb = bpool.tile([2, TB], f32, tag="ssb",
                                         name=f"ssb{ci}", bufs=5)
                        nc.vector.tensor_copy(ssb[:], ssp[0:2, :])
                        st[f"ssb{ci}"] = ssb
                    return f

                for ci in range(NQC + 1):  # 4 q chunks then k
                    w_sb = wq_sb if ci < NQC else wk_sb
                    fsl = slice(ci * P, (ci + 1) * P) if ci < NQC else slice(0, FKV)
                    ops.append(start_chunk(ci))
                    for dcc in range(NDC):
                        ops.append(mm_chunk(ci, dcc, w_sb, fsl))
                    ops.append(end_chunk(ci))

                # V^T: out[tok, feat] per 128-token chunk, accumulated over dc
                def start_v():
                    vp = psum.tile([P, TB], f32, tag="acc", name="vpack", bufs=2)
                    st["vpack"] = vp

                ops.append(start_v)

                def mm_v(tcc, dcc):
                    def f():
                        nc.tensor.matmul(
                            st["vpack"][:, tcc * P:(tcc + 1) * P],
                            st["xt"][:, dcc, tcc * P:(tcc + 1) * P],
                            wv_sb[:, dcc, :],
                            start=(dcc == 0), stop=(dcc == NDC - 1))
                    return f

                for tcc in range(NTC):
                    for dcc in range(NDC):
                        ops.append(mm_v(tcc, dcc))

                def end_v():
                    vp = st["vpack"]
                    for tcc in range(NTC):
                        kc = tbn * NTC + tcc
                        nc.vector.tensor_copy(v_sb[:, kc, 0, 0:64],
                                              vp[:, tcc * P:tcc * P + 64])
                        nc.vector.tensor_copy(v_sb[:, kc, 1, 0:64],
                                              vp[:, tcc * P + 64:(tcc + 1) * P])

                ops.append(end_v)
                return st, ops

            # ---------------------------------------------------------------
            # B-rope: grouped rsqrt + RoPE for block tbn (after feed drained)
            # ---------------------------------------------------------------
            qts_cur = {}

            def emit_rope_pre(tbn, st):
                # grouped sqrt(ms + eps) on ACT, then reciprocal on DVE
                rrs = []
                for ci in range(NQC + 1):
                    sst = bpool.tile([2, TB], f32, tag="sst", name=f"sst{ci}", bufs=5)
                    nc.scalar.activation(sst[:], st[f"ssb{ci}"][:],
                                         AF.Sqrt, bias=EPS, scale=1.0 / HD)
                    rrs.append(sst)
                rcs = []
                for ci in range(NQC + 1):
                    rr = bpool.tile([2, TB], f32, tag="rr", name=f"rr{ci}", bufs=5)
                    nc.vector.reciprocal_approx_fast(rr[:], rrs[ci][:])
                    rrb = bpool.tile([2, TB], bf16, tag="rrb", name=f"rrb{ci}",
                                     bufs=5)
                    nc.vector.tensor_copy(rrb[:], rr[:])
                    rcs.append(rrb)
                st["rcs"] = rcs

            def emit_rope(tbn, st):
                tbs = slice(tbn * TB, (tbn + 1) * TB)
                rcs = st["rcs"]

                for ci in range(NQC + 1):
                    is_k = ci == NQC
                    rT = rkT_sb if is_k else rqT_sb
                    ct = st["ck"] if is_k else st["cq"]
                    sn_t = st["sn"]
                    qsb = st[f"qsb{ci}"]
                    bc = psum.tile([P, TB], f32, tag="sps", name=f"bc{ci}", bufs=4)
                    nc.tensor.matmul(bc[:], hexp_sb[:], rcs[ci][:],
                                     start=True, stop=True)
                    qn = bpool.tile([P, TB], bf16, tag="qn", name=f"qn{ci}", bufs=3)
                    nc.vector.tensor_tensor(qn[:], qsb[:], bc[:], ALU.mult)
                    rot = psum.tile([P, TB], f32, tag="sps", name=f"rot{ci}", bufs=4)
                    nc.tensor.matmul(rot[:], rT[:], qn[:], start=True, stop=True)
                    m1 = bpool.tile([P, TB], bf16, tag="m1", name=f"m1_{ci}", bufs=2)
                    nc.vector.tensor_tensor(m1[:], qn[:], ct[:], ALU.mult)
                    m2 = bpool.tile([P, TB], bf16, tag="m2", name=f"m2_{ci}", bufs=2)
                    nc.vector.tensor_tensor(m2[:], rot[:], sn_t[:], ALU.mult)
                    if not is_k:
                        qt = qpool.tile([P, TB], bf16, tag=f"qt{ci}", name=f"qt{ci}")
                        nc.vector.tensor_tensor(qt[:], m1[:], m2[:], ALU.add)
                        qts_cur[ci] = qt
                    else:
                        nc.vector.tensor_tensor(ktf[:, tbs], m1[:], m2[:], ALU.add)
                        nc.vector.tensor_copy(kts[0:64, tbs], ktf[64:P, tbs])
                        nc.vector.tensor_copy(kts[64:P, tbs], ktf[0:64, tbs])

            # ---------------------------------------------------------------
            # D: attention for query block tb, draining `feed` into PE gaps
            # ---------------------------------------------------------------
            def emit_D(tb, feed_ops, e_ops, rope_hook=None):
                nkc = (tb + 1) * NTC
                n_iters = KVL * GROUPS * nkc
                qts = dict(qts_cur)   # rope_hook rebinds qts_cur for tb+1
                fi = 0
                ei = 0
                it = 0

                def drain(n):
                    nonlocal fi
                    for _ in range(n):
                        if fi < len(feed_ops):
                            feed_ops[fi]()
                            fi += 1

                e_every = max(1, n_iters // (len(e_ops) + 1)) if e_ops else 0

                def tick():
                    nonlocal ei, it
                    it += 1
                    if e_ops and ei < len(e_ops) and it % e_every == 0:
                        e_ops[ei]()
                        ei += 1

                # DMAs (xt + trig) issue immediately; front-load the rest so
                # the feed completes ~60% through D
                drain(2)
                per = ((len(feed_ops) + int(n_iters * 0.6)) // max(1, int(n_iters * 0.6))
                       if n_iters else 0)

                for g in range(KVL):
                    for pj in range(2):
                        if g == 1 and pj == 0 and rope_hook is not None:
                            drain(len(feed_ops))
                            rope_hook[0]()
                        if g == 1 and pj == 1 and rope_hook is not None:
                            rope_hook[1]()
                        o_pair = []
                        for hh in range(2):
                            hl = GROUPS * g + 2 * pj + hh
                            bq = 64 * (hl % 2)
                            cf = hl // 2
                            kt_tile = ktf if bq == 64 * g else kts
                            o_ps = psum.tile([P, TB], f32, tag="ops",
                                             name=f"ops{hl}", bufs=2)
                            o_pair.append(o_ps)
                            # diagonal band FIRST: batch 4 chunks of exp into
                            # esd, one affine_select, then the 4 deferred AVs.
                            # (diag-first keeps the GpSimd select latency off
                            # the head's accumulation tail)
                            esd = epool.tile([P, NTC, TB], bf16, tag="esd",
                                             name="esd", bufs=3)
                            for tdiag in range(NTC):
                                kc = tb * NTC + tdiag
                                ksl = slice(kc * P, (kc + 1) * P)
                                sps = psum.tile([P, TB], f32, tag="sps",
                                                name="sps", bufs=4)
                                nc.tensor.matmul(sps[:], kt_tile[bq:bq + 64, ksl],
                                                 qts[cf][bq:bq + 64, :],
                                                 start=True, stop=True)
                                nc.scalar.activation(esd[:, tdiag, :], sps[:],
                                                     AF.Exp, scale=float(SCALE))
                                drain(per)
                                tick()
                            # keep esd[p, t, f] where f >= p + 128*t
                            nc.gpsimd.affine_select(
                                out=esd[:], in_=esd[:],
                                pattern=[[-P, NTC], [1, TB]],
                                compare_op=ALU.is_ge, fill=0.0,
                                base=0, channel_multiplier=-1)
                            for tdiag in range(NTC):
                                kc = tb * NTC + tdiag
                                nc.tensor.matmul(o_ps[0:65, :],
                                                 v_sb[:, kc, g, 0:65],
                                                 esd[:, tdiag, :],
                                                 start=(tdiag == 0),
                                                 stop=(tb == 0 and tdiag == NTC - 1))
                            # off-diagonal blocks: exp straight to AV
                            for kc in range(tb * NTC):
                                ksl = slice(kc * P, (kc + 1) * P)
                                sps = psum.tile([P, TB], f32, tag="sps",
                                                name="sps", bufs=4)
                                nc.tensor.matmul(sps[:], kt_tile[bq:bq + 64, ksl],
                                                 qts[cf][bq:bq + 64, :],
                                                 start=True, stop=True)
                                es = epool.tile([P, TB], bf16, tag="es", name="es",
                                                bufs=8)
                                nc.scalar.activation(es[:], sps[:], AF.Exp,
                                                     scale=float(SCALE))
                                nc.tensor.matmul(o_ps[0:65, :],
                                                 v_sb[:, kc, g, 0:65], es[:],
                                                 start=False,
                                                 stop=(kc == tb * NTC - 1))
                                drain(per)
                                tick()
                        # normalize pair -> orhs[cf2]
                        cf2 = 2 * g + pj
                        dnA = npool.tile([1, TB], f32, tag="dn", name="dnA", bufs=4)
                        dnB = npool.tile([1, TB], f32, tag="dn", name="dnB", bufs=4)
                        nc.vector.tensor_copy(dnA[:], o_pair[0][64:65, :])
                        nc.vector.tensor_copy(dnB[:], o_pair[1][64:65, :])
                        rpA = npool.tile([1, TB], f32, tag="rp", name="rpA", bufs=4)
                        rpB = npool.tile([1, TB], f32, tag="rp", name="rpB", bufs=4)
                        nc.vector.reciprocal_approx_fast(rpA[:], dnA[:])
                        nc.vector.reciprocal_approx_fast(rpB[:], dnB[:])
                        rpAb = npool.tile([1, TB], bf16, tag="rpb", name="rpAb",
                                          bufs=4)
                        rpBb = npool.tile([1, TB], bf16, tag="rpb", name="rpBb",
                                          bufs=4)
                        nc.vector.tensor_copy(rpAb[:], rpA[:])
                        nc.vector.tensor_copy(rpBb[:], rpB[:])
                        bc2 = psum.tile([P, TB], f32, tag="sps", name="bc2",
                                        bufs=4)
                        nc.tensor.matmul(bc2[:], hexp_sb[0:1, :], rpAb[:],
                                         start=True, stop=False)
                        nc.tensor.matmul(bc2[:], e1_sb[:], rpBb[:],
                                         start=False, stop=True)
                        osb = npool.tile([P, TB], bf16, tag="osb", name="osb")
                        nc.vector.tensor_copy(osb[0:64, :], o_pair[0][0:64, :])
                        nc.vector.tensor_copy(osb[64:P, :], o_pair[1][0:64, :])
                        orhs = opool.tile([P, TB], bf16, tag=f"orhs{cf2}",
                                          name=f"orhs{cf2}")
                        nc.vector.tensor_tensor(orhs[:], osb[:], bc2[:], ALU.mult)
                        if cf2 == 0:
                            orhs_l = [None] * NQC
                            st_orhs[0] = orhs_l
                        st_orhs[0][cf2] = orhs
                drain(len(feed_ops))
                while e_ops and ei < len(e_ops):
                    e_ops[ei]()
                    ei += 1

            st_orhs = [None]

            # ---------------------------------------------------------------
            # E: output projection for block tb
            # ---------------------------------------------------------------
            def make_E(tb):
                tbs = slice(tb * TB, (tb + 1) * TB)
                orhs_l = st_orhs[0]

                def blk(dc2):
                    def f():
                        ops_ = psum.tile([P, TB], f32, tag="acc", name="ops_",
                                         bufs=2)
                        for cf in range(NQC):
                            nc.tensor.matmul(
                                ops_[:], wo_sb[:, cf, dc2 * P:(dc2 + 1) * P],
                                orhs_l[cf][:], start=(cf == 0),
                                stop=(cf == NQC - 1))
                        ob = outp.tile([P, TB], f32, tag="ob", name="ob")
                        nc.vector.tensor_copy(ob[:], ops_[:])
                        nc.gpsimd.dma_start(outT_d[dc2 * P:(dc2 + 1) * P, tbs],
                                            ob[:])
                    return f

                return [blk(dc2) for dc2 in range(NDC)]

            # ---------------------------------------------------------------
            # main schedule: D(tb) drains the A/B feed for tb+1 plus the
            # output projection of tb-1 into the PE's exp-wait gaps.
            # ---------------------------------------------------------------
            st0, feed0 = make_feed(0)
            for op in feed0:
                op()
            emit_rope_pre(0, st0)
            emit_rope(0, st0)
            e_prev = []
            for tb in range(NTB):
                if tb + 1 < NTB:
                    st_next, feed_next = make_feed(tb + 1)
                    hook = (lambda s=st_next, t=tb + 1: emit_rope_pre(t, s),
                            lambda s=st_next, t=tb + 1: emit_rope(t, s))
                else:
                    st_next, feed_next, hook = None, [], None
                emit_D(tb, feed_next, e_prev, rope_hook=hook)
                e_prev = make_E(tb)
            for op in e_prev:
                op()

    nc.compile()
    return nc


_NC_CACHE = None


def _get_nc():
    global _NC_CACHE
    if _NC_CACHE is None:
        _NC_CACHE = _build_nc()
    return _NC_CACHE


def _host_constants(q_scale, k_scale):
    pos = np.arange(T, dtype=np.float64)
    invf = 1.0 / (THETA ** (np.arange(0, HD, 2, dtype=np.float64) / HD))  # (32,)
    ang = pos[:, None] * invf[None, :]                                    # (T, 32)
    c = np.cos(ang)
    s = np.sin(ang)
    pidx = np.arange(P) % 32
    hidx = np.arange(P) % HD
    cosq = (c[:, pidx].T * q_scale[hidx][:, None]).astype(ml_dtypes.bfloat16)
    cosk = (c[:, pidx].T * k_scale[hidx][:, None]).astype(ml_dtypes.bfloat16)
    sin = s[:, pidx].T.astype(ml_dtypes.bfloat16)

    def rmat(scale):
        R = np.zeros((HD, HD), dtype=np.float64)
        for i in range(32):
            R[i, i + 32] = -scale[i + 32]
            R[i + 32, i] = scale[i]
        M = np.kron(np.eye(2), R)
        return np.ascontiguousarray(M.T.astype(ml_dtypes.bfloat16))

    hsel = np.zeros((P, 2), dtype=ml_dtypes.bfloat16)
    hsel[0:64, 0] = 1.0
    hsel[64:P, 1] = 1.0
    hexp = np.zeros((2, P), dtype=ml_dtypes.bfloat16)
    hexp[0, 0:64] = 1.0
    hexp[1, 64:P] = 1.0

    return cosq, cosk, sin, rmat(q_scale), rmat(k_scale), hsel, hexp


def _run(inputs, trace=False):
    x = np.asarray(inputs["x"], dtype=np.float32)
    Wq = np.asarray(inputs["Wq"], dtype=np.float32)
    Wk = np.asarray(inputs["Wk"], dtype=np.float32)
    Wv = np.asarray(inputs["Wv"], dtype=np.float32)
    Wo = np.asarray(inputs["Wo"], dtype=np.float32)
    q_scale = np.asarray(inputs["q_scale"], dtype=np.float64)
    k_scale = np.asarray(inputs["k_scale"], dtype=np.float64)

    cosq, cosk, sin, rqT, rkT, hsel, hexp = _host_constants(q_scale, k_scale)

    bf = ml_dtypes.bfloat16
    in_maps = []
    for c in range(8):
        b = c // 4
        r = c % 4
        in_maps.append({
            "xT": np.ascontiguousarray(x[b].T).astype(bf),
            "wq": np.ascontiguousarray(Wq[:, r * FQ:(r + 1) * FQ]).astype(bf),
            "wk": np.ascontiguousarray(Wk[:, r * FKV:(r + 1) * FKV]).astype(bf),
            "wv": np.ascontiguousarray(Wv[:, r * FKV:(r + 1) * FKV]).astype(bf),
            "wo": np.ascontiguousarray(Wo[r * FQ:(r + 1) * FQ, :]).astype(bf),
            "cosq": cosq, "cosk": cosk, "sin": sin,
            "rqT": rqT, "rkT": rkT, "hsel": hsel,
            "hexp": hexp, "e1": np.ascontiguousarray(hexp[1:2, :]),
        })

    nc = _get_nc()
    res = run_bass_kernel_spmd(nc, in_maps, core_ids=list(range(8)), trace=trace)
    out = np.empty((B, T, D), dtype=np.float32)
    for b in range(B):
        acc = res.results[4 * b]["outT"].astype(np.float32).copy()
        for r in range(1, 4):
            acc += res.results[4 * b + r]["outT"]
        out[b] = acc.T
    return out, res


def kernel(**inputs):
    out, _ = _run(inputs, trace=False)
    return out


# revision 50
# speedup vs baseline: 1.8817x; 1.0334x over previous
"""GQA (32 q heads / 8 kv heads, head_dim 64, causal, QK-RMSNorm + RoPE) on 8 TRN2 cores.

Sharding: data-parallel over batch (2) x tensor-parallel over heads (4):
each core handles one batch element, 8 query heads, 2 kv heads, and produces
a partial output (its heads' slice of the Wo contraction); the host sums the
4 partials per batch element.

v2: all matmuls in bf16 (PSUM accumulation stays f32), software-pipelined so
the PE never idles (projection matmuls for block tb+1 are interleaved into
the attention inner loop of block tb), RMSNorm via Sqrt+DVE-reciprocal
(no Ln/Exp table thrash), causal masks applied on GpSimd, V^T produced
directly by the projection (no PE transposes).
"""

import numpy as np
import ml_dtypes

import concourse.bass as bass
import concourse.mybir as mybir
import concourse.tile as tile
from concourse import bacc
from concourse.bass_utils import run_bass_kernel_spmd

# Problem config (hardcoded per contract)
B, T, D = 2, 2048, 2048
H, KV, HD = 32, 8, 64
GROUPS = H // KV
THETA = 10000.0
SCALE = 1.0 / np.sqrt(HD)
EPS = 1e-6

# Per-core sharding
HQL = H // 4          # 8 local q heads
KVL = KV // 4         # 2 local kv heads
FQ = HQL * HD         # 512
FKV = KVL * HD        # 128

# Tiling
P = 128
TB = 512              # token block
NTB = T // TB         # 4
NDC = D // P          # 16 contraction chunks
NKC = T // P          # 16 key chunks
NQC = FQ // P         # 4 q-proj chunks (2 heads each)
NTC = TB // P         # 4 token chunks per block

f32 = mybir.dt.float32
bf16 = mybir.dt.bfloat16
AF = mybir.ActivationFunctionType
ALU = mybir.AluOpType


def _build_nc():
    nc = bacc.Bacc("TRN2", target_bir_lowering=False, debug=False, num_devices=8)

    eps_t = nc.alloc_sbuf_tensor("const-f32-eps", [128, 1], f32)
    nc.gpsimd.memset(eps_t.ap(), EPS)
    nc.const_aps.aps[(f32, EPS)] = eps_t.ap()
    zero_t = nc.alloc_sbuf_tensor("const-f32-zero", [128, 1], f32)
    nc.gpsimd.memset(zero_t.ap(), 0.0)
    nc.const_aps.aps[(f32, 0.0)] = zero_t.ap()
    nc.all_engine_barrier()

    xT_d = nc.dram_tensor("xT", [D, T], bf16, kind="ExternalInput")
    wq_d = nc.dram_tensor("wq", [D, FQ], bf16, kind="ExternalInput")
    wk_d = nc.dram_tensor("wk", [D, FKV], bf16, kind="ExternalInput")
    wv_d = nc.dram_tensor("wv", [D, FKV], bf16, kind="ExternalInput")
    wo_d = nc.dram_tensor("wo", [FQ, D], bf16, kind="ExternalInput")
    cosq_d = nc.dram_tensor("cosq", [P, T], bf16, kind="ExternalInput")
    cosk_d = nc.dram_tensor("cosk", [P, T], bf16, kind="ExternalInput")
    sin_d = nc.dram_tensor("sin", [P, T], bf16, kind="ExternalInput")
    rqT_d = nc.dram_tensor("rqT", [P, P], bf16, kind="ExternalInput")
    rkT_d = nc.dram_tensor("rkT", [P, P], bf16, kind="ExternalInput")
    hsel_d = nc.dram_tensor("hsel", [P, 2], bf16, kind="ExternalInput")
    hexp_d = nc.dram_tensor("hexp", [2, P], bf16, kind="ExternalInput")
    e1_d = nc.dram_tensor("e1", [1, P], bf16, kind="ExternalInput")
    outT_d = nc.dram_tensor("outT", [D, T], f32, kind="ExternalOutput")

    with tile.TileContext(nc) as tc:
        with (
            tc.tile_pool(name="wpool", bufs=1) as wpool,
            tc.tile_pool(name="cpool", bufs=1) as cpool,
            tc.tile_pool(name="kvpool", bufs=1) as kvpool,
            tc.tile_pool(name="xpool", bufs=2) as xpool,
            tc.tile_pool(name="trig", bufs=2) as trig,
            tc.tile_pool(name="bpool", bufs=6) as bpool,
            tc.tile_pool(name="qpool", bufs=2) as qpool,
            tc.tile_pool(name="epool", bufs=6) as epool,
            tc.tile_pool(name="npool", bufs=2) as npool,
            tc.tile_pool(name="opool", bufs=2) as opool,
            tc.tile_pool(name="outp", bufs=2) as outp,
            tc.tile_pool(name="psum", bufs=1, space="PSUM") as psum,
        ):
            # ---- persistent weights / constants ----
            wq_sb = wpool.tile([P, NDC, FQ], bf16)
            wk_sb = wpool.tile([P, NDC, FKV], bf16)
            wv_sb = wpool.tile([P, NDC, FKV], bf16)
            wo_sb = wpool.tile([P, NQC, D], bf16)
            nc.sync.dma_start(wq_sb[:], wq_d.rearrange("(ko p) f -> p ko f", p=P))
            nc.sync.dma_start(wk_sb[:], wk_d.rearrange("(ko p) f -> p ko f", p=P))
            nc.sync.dma_start(wv_sb[:], wv_d.rearrange("(ko p) f -> p ko f", p=P))
            nc.sync.dma_start(wo_sb[:], wo_d.rearrange("(ko p) f -> p ko f", p=P))

            rqT_sb = cpool.tile([P, P], bf16)
            rkT_sb = cpool.tile([P, P], bf16)
            hsel_sb = cpool.tile([P, 2], bf16)
            hexp_sb = cpool.tile([2, P], bf16)
            e1_sb = cpool.tile([1, P], bf16)
            nc.sync.dma_start(rqT_sb[:], rqT_d[:])
            nc.sync.dma_start(rkT_sb[:], rkT_d[:])
            nc.sync.dma_start(hsel_sb[:], hsel_d[:])
            nc.sync.dma_start(hexp_sb[:], hexp_d[:])
            nc.sync.dma_start(e1_sb[:], e1_d[:])

            # K^T per-kv-head at both partition placements, V (+ones col)
            ktf = kvpool.tile([P, T], bf16)          # rows 0:64 kv0, 64:128 kv1
            kts = kvpool.tile([P, T], bf16)          # swapped halves
            v_sb = kvpool.tile([P, NKC, KVL, 66], bf16)  # [tok, kc, g, hd+ones+pad]
            ones_bc = nc.const_aps.tensor(1.0, (P, NKC, KVL, 66), bf16)
            nc.vector.tensor_copy(v_sb[:], ones_bc)

            # ---------------------------------------------------------------
            # Feed: projection + square/copy work for token block tbn,
            # returned as a list of closures to be drained into D(tbn-1).
            # ---------------------------------------------------------------
            def make_feed(tbn):
                tbs = slice(tbn * TB, (tbn + 1) * TB)
                st = {}
                ops = []

                def load_xt():
                    xt = xpool.tile([P, NDC, TB], bf16, tag="xt", name="xt",
                                    bufs=2)
                    xr = xT_d.rearrange("(ko p) t -> p ko t", p=P)
                    for q4 in range(4):
                        nc.sync.dma_start(xt[:, 4 * q4:4 * (q4 + 1), :],
                                          xr[:, 4 * q4:4 * (q4 + 1), tbs])
                    st["xt"] = xt

                ops.append(load_xt)

                def load_trig():
                    cq_t = trig.tile([P, TB], bf16, tag="cq", name="cq_t")
                    ck_t = trig.tile([P, TB], bf16, tag="ck", name="ck_t")
                    sn_t = trig.tile([P, TB], bf16, tag="sn", name="sn_t")
                    nc.sync.dma_start(cq_t[:], cosq_d[:, tbs])
                    nc.sync.dma_start(ck_t[:], cosk_d[:, tbs])
                    nc.sync.dma_start(sn_t[:], sin_d[:, tbs])
                    st["cq"], st["ck"], st["sn"] = cq_t, ck_t, sn_t

                ops.append(load_trig)

                # q0..q3 and k projection chunks (chunk-major over dc)
                def start_chunk(ci):
                    def f():
                        acc = psum.tile([P, TB], f32, tag="acc", name=f"acc{ci}",
                                        bufs=2)
                        st["acc"] = acc
                    return f

                def mm_chunk(ci, dcc, w_sb, fsl):
                    def f():
                        nc.tensor.matmul(st["acc"][:], w_sb[:, dcc, fsl],
                                         st["xt"][:, dcc, :],
                                         start=(dcc == 0), stop=(dcc == NDC - 1))
                    return f

                def end_chunk(ci):
                    # bf16 copy for RoPE, square on DVE, packed sumsq matmul
                    def f():
                        acc = st["acc"]
                        qsb = bpool.tile([P, TB], bf16, tag="qsb", name=f"qsb{ci}")
                        nc.vector.tensor_copy(qsb[:], acc[:])
                        st[f"qsb{ci}"] = qsb
                        sq = bpool.tile([P, TB], bf16, tag="sq", name=f"sq{ci}")
                        nc.vector.tensor_tensor(sq[:], qsb[:], qsb[:], ALU.mult)
                        ssp = psum.tile([P, TB], f32, tag="sps", name="ssp",
                                        bufs=4)
                        nc.tensor.matmul(ssp[0:2, :], hsel_sb[:], sq[:],
                                         start=True, stop=True)
                        ssb = bpool.tile([2, TB], f32, tag="ssb",
                                         name=f"ssb{ci}", bufs=5)
                        nc.vector.tensor_copy(ssb[:], ssp[0:2, :])
                        st[f"ssb{ci}"] = ssb
                    return f

                for ci in range(NQC + 1):  # 4 q chunks then k
                    w_sb = wq_sb if ci < NQC else wk_sb
                    fsl = slice(ci * P, (ci + 1) * P) if ci < NQC else slice(0, FKV)
                    ops.append(start_chunk(ci))
                    for dcc in range(NDC):
                        ops.append(mm_chunk(ci, dcc, w_sb, fsl))
                    ops.append(end_chunk(ci))

                # V^T: out[tok, feat] per 128-token chunk, accumulated over dc
                def start_v():
                    vp = psum.tile([P, TB], f32, tag="acc", name="vpack", bufs=2)
                    st["vpack"] = vp

                ops.append(start_v)

                def mm_v(tcc, dcc):
                    def f():
                        nc.tensor.matmul(
                            st["vpack"][:, tcc * P:(tcc + 1) * P],
                            st["xt"][:, dcc, tcc * P:(tcc + 1) * P],
                            wv_sb[:, dcc, :],
                            start=(dcc == 0), stop=(dcc == NDC - 1))
                    return f

                for tcc in range(NTC):
                    for dcc in range(NDC):
                        ops.append(mm_v(tcc, dcc))

                def end_v():
                    vp = st["vpack"]
                    for tcc in range(NTC):
                        kc = tbn * NTC + tcc
                        nc.vector.tensor_copy(v_sb[:, kc, 0, 0:64],
                                              vp[:, tcc * P:tcc * P + 64])
                        nc.vector.tensor_copy(v_sb[:, kc, 1, 0:64],
                                              vp[:, tcc * P + 64:(tcc + 1) * P])

                ops.append(end_v)
                return st, ops

            # ---------------------------------------------------------------
            # B-rope: grouped rsqrt + RoPE for block tbn (after feed drained)
            # ---------------------------------------------------------------
            qts_cur = {}

            def emit_rope_pre(tbn, st):
                # grouped sqrt(ms + eps) on ACT, then reciprocal on DVE
                rrs = []
                for ci in range(NQC + 1):
                    sst = bpool.tile([2, TB], f32, tag="sst", name=f"sst{ci}", bufs=2)
                    nc.scalar.activation(sst[:], st[f"ssb{ci}"][:],
                                         AF.Sqrt, bias=EPS, scale=1.0 / HD)
                    rrs.append(sst)
                rcs = []
                for ci in range(NQC + 1):
                    rr = bpool.tile([2, TB], f32, tag="rr", name=f"rr{ci}", bufs=2)
                    nc.vector.reciprocal_approx_fast(rr[:], rrs[ci][:])
                    rrb = bpool.tile([2, TB], bf16, tag="rrb", name=f"rrb{ci}",
                                     bufs=5)
                    nc.vector.tensor_copy(rrb[:], rr[:])
                    rcs.append(rrb)
                st["rcs"] = rcs

            def emit_rope(tbn, st):
                tbs = slice(tbn * TB, (tbn + 1) * TB)
                rcs = st["rcs"]

                for ci in range(NQC + 1):
                    is_k = ci == NQC
                    rT = rkT_sb if is_k else rqT_sb
                    ct = st["ck"] if is_k else st["cq"]
                    sn_t = st["sn"]
                    qsb = st[f"qsb{ci}"]
                    bc = psum.tile([P, TB], f32, tag="sps", name=f"bc{ci}", bufs=4)
                    nc.tensor.matmul(bc[:], hexp_sb[:], rcs[ci][:],
                                     start=True, stop=True)
                    qn = bpool.tile([P, TB], bf16, tag="qn", name=f"qn{ci}", bufs=3)
                    nc.vector.tensor_tensor(qn[:], qsb[:], bc[:], ALU.mult)
                    rot = psum.tile([P, TB], f32, tag="sps", name=f"rot{ci}", bufs=4)
                    nc.tensor.matmul(rot[:], rT[:], qn[:], start=True, stop=True)
                    m1 = bpool.tile([P, TB], bf16, tag="m1", name=f"m1_{ci}", bufs=2)
                    nc.vector.tensor_tensor(m1[:], qn[:], ct[:], ALU.mult)
                    m2 = bpool.tile([P, TB], bf16, tag="m2", name=f"m2_{ci}", bufs=2)
                    nc.vector.tensor_tensor(m2[:], rot[:], sn_t[:], ALU.mult)
                    if not is_k:
                        qt = qpool.tile([P, TB], bf16, tag=f"qt{ci}", name=f"qt{ci}")
                        nc.vector.tensor_tensor(qt[:], m1[:], m2[:], ALU.add)
                        qts_cur[ci] = qt
                    else:
                        nc.vector.tensor_tensor(ktf[:, tbs], m1[:], m2[:], ALU.add)
                        nc.vector.tensor_copy(kts[0:64, tbs], ktf[64:P, tbs])
                        nc.vector.tensor_copy(kts[64:P, tbs], ktf[0:64, tbs])

            # ---------------------------------------------------------------
            # D: attention for query block tb, draining `feed` into PE gaps
            # ---------------------------------------------------------------
            def emit_D(tb, feed_ops, e_ops, rope_hook=None):
                nkc = (tb + 1) * NTC
                n_iters = KVL * GROUPS * nkc
                qts = dict(qts_cur)   # rope_hook rebinds qts_cur for tb+1
                fi = 0
                ei = 0
                it = 0

                def drain(n):
                    nonlocal fi
                    for _ in range(n):
                        if fi < len(feed_ops):
                            feed_ops[fi]()
                            fi += 1

                e_every = max(1, n_iters // (len(e_ops) + 1)) if e_ops else 0

                def tick():
                    nonlocal ei, it
                    it += 1
                    if e_ops and ei < len(e_ops) and it % e_every == 0:
                        e_ops[ei]()
                        ei += 1

                # DMAs (xt + trig) issue immediately; front-load the rest so
                # the feed completes ~60% through D
                drain(2)
                per = ((len(feed_ops) + int(n_iters * 0.6)) // max(1, int(n_iters * 0.6))
                       if n_iters else 0)

                for g in range(KVL):
                    for pj in range(2):
                        if g == 1 and pj == 0 and rope_hook is not None:
                            drain(len(feed_ops))
                            rope_hook[0]()
                        if g == 1 and pj == 1 and rope_hook is not None:
                            rope_hook[1]()
                        o_pair = []
                        for hh in range(2):
                            hl = GROUPS * g + 2 * pj + hh
                            bq = 64 * (hl % 2)
                            cf = hl // 2
                            kt_tile = ktf if bq == 64 * g else kts
                            o_ps = psum.tile([P, TB], f32, tag="ops",
                                             name=f"ops{hl}", bufs=2)
                            o_pair.append(o_ps)
                            # diagonal band FIRST: batch 4 chunks of exp into
                            # esd, one affine_select, then the 4 deferred AVs.
                            # (diag-first keeps the GpSimd select latency off
                            # the head's accumulation tail)
                            esd = epool.tile([P, NTC, TB], bf16, tag="esd",
                                             name="esd", bufs=3)
                            for tdiag in range(NTC):
                                kc = tb * NTC + tdiag
                                ksl = slice(kc * P, (kc + 1) * P)
                                sps = psum.tile([P, TB], f32, tag="sps",
                                                name="sps", bufs=4)
                                nc.tensor.matmul(sps[:], kt_tile[bq:bq + 64, ksl],
                                                 qts[cf][bq:bq + 64, :],
                                                 start=True, stop=True)
                                nc.scalar.activation(esd[:, tdiag, :], sps[:],
                                                     AF.Exp, scale=float(SCALE))
                                drain(per)
                                tick()
                            # keep esd[p, t, f] where f >= p + 128*t
                            nc.gpsimd.affine_select(
                                out=esd[:], in_=esd[:],
                                pattern=[[-P, NTC], [1, TB]],
                                compare_op=ALU.is_ge, fill=0.0,
                                base=0, channel_multiplier=-1)
                            for tdiag in range(NTC):
                                kc = tb * NTC + tdiag
                                nc.tensor.matmul(o_ps[0:65, :],
                                                 v_sb[:, kc, g, 0:65],
                                                 esd[:, tdiag, :],
                                                 start=(tdiag == 0),
                                                 stop=(tb == 0 and tdiag == NTC - 1))
                            # off-diagonal blocks: exp straight to AV
                            for kc in range(tb * NTC):
                                ksl = slice(kc * P, (kc + 1) * P)
                                sps = psum.tile([P, TB], f32, tag="sps",
                                                name="sps", bufs=4)
                                nc.tensor.matmul(sps[:], kt_tile[bq:bq + 64, ksl],
                                                 qts[cf][bq:bq + 64, :],
                                                 start=True, stop=True)
                                es = epool.tile([P, TB], bf16, tag="es", name="es",
                                                bufs=8)
                                nc.scalar.activation(es[:], sps[:], AF.Exp,
                                                     scale=float(SCALE))
                                nc.tensor.matmul(o_ps[0:65, :],
                                                 v_sb[:, kc, g, 0:65], es[:],
                                                 start=False,
                                                 stop=(kc == tb * NTC - 1))
                                drain(per)
                                tick()
                        # normalize pair -> orhs[cf2]
                        cf2 = 2 * g + pj
                        dnA = npool.tile([1, TB], f32, tag="dn", name="dnA",
                                         bufs=4)
                        dnB = npool.tile([1, TB], f32, tag="dn", name="dnB",
                                         bufs=4)
                        nc.vector.tensor_copy(dnA[:], o_pair[0][64:65, :])
                        nc.vector.tensor_copy(dnB[:], o_pair[1][64:65, :])
                        rpA = npool.tile([1, TB], f32, tag="rp", name="rpA",
                                         bufs=4)
                        rpB = npool.tile([1, TB], f32, tag="rp", name="rpB",
                                         bufs=4)
                        nc.vector.reciprocal_approx_fast(rpA[:], dnA[:])
                        nc.vector.reciprocal_approx_fast(rpB[:], dnB[:])
                        rpAb = npool.tile([1, TB], bf16, tag="rpb", name="rpAb",
                                          bufs=4)
                        rpBb = npool.tile([1, TB], bf16, tag="rpb", name="rpBb",
                                          bufs=4)
                        nc.vector.tensor_copy(rpAb[:], rpA[:])
                        nc.vector.tensor_copy(rpBb[:], rpB[:])
                        bc2 = psum.tile([P, TB], f32, tag="sps", name="bc2",
                                        bufs=4)
                        nc.tensor.matmul(bc2[:], hexp_sb[0:1, :], rpAb[:],
                                         start=True, stop=False)
                        nc.tensor.matmul(bc2[:], e1_sb[:], rpBb[:],
                                         start=False, stop=True)
                        osb = npool.tile([P, TB], bf16, tag="osb", name="osb")
                        nc.vector.tensor_copy(osb[0:64, :], o_pair[0][0:64, :])
                        nc.vector.tensor_copy(osb[64:P, :], o_pair[1][0:64, :])
                        orhs = opool.tile([P, TB], bf16, tag=f"orhs{cf2}",
                                          name=f"orhs{cf2}")
                        nc.vector.tensor_tensor(orhs[:], osb[:], bc2[:], ALU.mult)
                        if cf2 == 0:
                            orhs_l = [None] * NQC
                            st_orhs[0] = orhs_l
                        st_orhs[0][cf2] = orhs
                drain(len(feed_ops))
                while e_ops and ei < len(e_ops):
                    e_ops[ei]()
                    ei += 1

            st_orhs = [None]

            # ---------------------------------------------------------------
            # E: output projection for block tb
            # ---------------------------------------------------------------
            def make_E(tb):
                tbs = slice(tb * TB, (tb + 1) * TB)
                orhs_l = st_orhs[0]

                def blk(dc2):
                    def f():
                        ops_ = psum.tile([P, TB], f32, tag="acc", name="ops_",
                                         bufs=2)
                        for cf in range(NQC):
                            nc.tensor.matmul(
                                ops_[:], wo_sb[:, cf, dc2 * P:(dc2 + 1) * P],
                                orhs_l[cf][:], start=(cf == 0),
                                stop=(cf == NQC - 1))
                        ob = outp.tile([P, TB], f32, tag="ob", name="ob",
                                       bufs=4)
                        if dc2 % 2 == 0:
                            nc.vector.tensor_copy(ob[:], ops_[:])
                        else:
                            nc.scalar.copy(ob[:], ops_[:])
                        nc.sync.dma_start(outT_d[dc2 * P:(dc2 + 1) * P, tbs],
                                          ob[:])
                    return f

                return [blk(dc2) for dc2 in range(NDC)]

            # ---------------------------------------------------------------
            # main schedule: D(tb) drains the A/B feed for tb+1 plus the
            # output projection of tb-1 into the PE's exp-wait gaps.
            # ---------------------------------------------------------------
            st0, feed0 = make_feed(0)
            for op in feed0:
                op()
            emit_rope_pre(0, st0)
            emit_rope(0, st0)
            e_prev = []
            for tb in range(NTB):
                if tb + 1 < NTB:
                    st_next, feed_next = make_feed(tb + 1)
                    hook = (lambda s=st_next, t=tb + 1: emit_rope_pre(t, s),
                            lambda s=st_next, t=tb + 1: emit_rope(t, s))
                else:
                    st_next, feed_next, hook = None, [], None
                emit_D(tb, feed_next, e_prev, rope_hook=hook)
                e_prev = make_E(tb)
            for op in e_prev:
                op()

    nc.compile()
    return nc


_NC_CACHE = None


def _get_nc():
    global _NC_CACHE
    if _NC_CACHE is None:
        _NC_CACHE = _build_nc()
    return _NC_CACHE


def _host_constants(q_scale, k_scale):
    pos = np.arange(T, dtype=np.float64)
    invf = 1.0 / (THETA ** (np.arange(0, HD, 2, dtype=np.float64) / HD))  # (32,)
    ang = pos[:, None] * invf[None, :]                                    # (T, 32)
    c = np.cos(ang)
    s = np.sin(ang)
    pidx = np.arange(P) % 32
    hidx = np.arange(P) % HD
    cosq = (c[:, pidx].T * q_scale[hidx][:, None]).astype(ml_dtypes.bfloat16)
    cosk = (c[:, pidx].T * k_scale[hidx][:, None]).astype(ml_dtypes.bfloat16)
    sin = s[:, pidx].T.astype(ml_dtypes.bfloat16)

    def rmat(scale):
        R = np.zeros((HD, HD), dtype=np.float64)
        for i in range(32):
            R[i, i + 32] = -scale[i + 32]
            R[i + 32, i] = scale[i]
        M = np.kron(np.eye(2), R)
        return np.ascontiguousarray(M.T.astype(ml_dtypes.bfloat16))

    hsel = np.zeros((P, 2), dtype=ml_dtypes.bfloat16)
    hsel[0:64, 0] = 1.0
    hsel[64:P, 1] = 1.0
    hexp = np.zeros((2, P), dtype=ml_dtypes.bfloat16)
    hexp[0, 0:64] = 1.0
    hexp[1, 64:P] = 1.0

    return cosq, cosk, sin, rmat(q_scale), rmat(k_scale), hsel, hexp


def _run(inputs, trace=False):
    x = np.asarray(inputs["x"], dtype=np.float32)
    Wq = np.asarray(inputs["Wq"], dtype=np.float32)
    Wk = np.asarray(inputs["Wk"], dtype=np.float32)
    Wv = np.asarray(inputs["Wv"], dtype=np.float32)
    Wo = np.asarray(inputs["Wo"], dtype=np.float32)
    q_scale = np.asarray(inputs["q_scale"], dtype=np.float64)
    k_scale = np.asarray(inputs["k_scale"], dtype=np.float64)

    cosq, cosk, sin, rqT, rkT, hsel, hexp = _host_constants(q_scale, k_scale)

    bf = ml_dtypes.bfloat16
    in_maps = []
    for c in range(8):
        b = c // 4
        r = c % 4
        in_maps.append({
            "xT": np.ascontiguousarray(x[b].T).astype(bf),
            "wq": np.ascontiguousarray(Wq[:, r * FQ:(r + 1) * FQ]).astype(bf),
            "wk": np.ascontiguousarray(Wk[:, r * FKV:(r + 1) * FKV]).astype(bf),
            "wv": np.ascontiguousarray(Wv[:, r * FKV:(r + 1) * FKV]).astype(bf),
            "wo": np.ascontiguousarray(Wo[r * FQ:(r + 1) * FQ, :]).astype(bf),
            "cosq": cosq, "cosk": cosk, "sin": sin,
            "rqT": rqT, "rkT": rkT, "hsel": hsel,
            "hexp": hexp, "e1": np.ascontiguousarray(hexp[1:2, :]),
        })

    nc = _get_nc()
    res = run_bass_kernel_spmd(nc, in_maps, core_ids=list(range(8)), trace=trace)
    out = np.empty((B, T, D), dtype=np.float32)
    for b in range(B):
        acc = res.results[4 * b]["outT"].astype(np.float32).copy()
        for r in range(1, 4):
            acc += res.results[4 * b + r]["outT"]
        out[b] = acc.T
    return out, res


def kernel(**inputs):
    out, _ = _run(inputs, trace=False)
    return out
